# revision 48
# baseline (speedup 1.0000x reference)
"""Bass/Tile TRN2 kernel for EnhancedIPA3 (invariant-point-attention variant).

Sharding: 8 cores = batch(2) x query-block(4).  NO collectives: each core
recomputes the full K/V side for its batch locally (deterministic; the
AllGather variant was hostage to inter-core launch skew).  Query side runs
on the core's own 256 rows (separately-sliced inputs).

Point channels are pair-interleaved [c0 d0 c1 d1 c2 d2] per point so each
rigid-transform madd covers coord+dir at double width, and the transforms
write straight into the attention feature tiles (kf/va/qf) -- no staging
buffers or assembly copies.  s arrives host-pretransposed in bf16.  All
biases are zero per the problem spec and are dropped.

Self-contained: hardcodes all shapes; only depends on numpy + concourse.
"""

import numpy as np
from contextlib import ExitStack

import concourse.bass as bass
import concourse.bacc as bacc
import concourse.mybir as mybir
import concourse.tile as tile
from concourse.bass_utils import run_bass_kernel_spmd
from concourse.masks import make_identity

F32 = mybir.dt.float32
F32R = mybir.dt.float32r
BF16 = mybir.dt.bfloat16
AF = mybir.ActivationFunctionType
OP = mybir.AluOpType

B, N, CS, H, C, P, V = 2, 1024, 384, 12, 16, 4, 8
EPS = 1e-8
NB = N // 4            # 256 query rows per core
NT = NB // 128
KT = N // 128          # 8 key row-tiles (full batch, local)
K_OFF, V_OFF, KVP_OFF = 0, 192, 384
Q_OFF, G_OFF, QP_OFF = 1248, 1440, 1488
WALL_COLS = 1776
FEAT = 64              # per-head feature stride in kf/qf
FS = 42
OCH = 66
FOUT = H * (C + 7 * V)
KCH = 7
NKB = 8


def _host_prep(inputs):
    wq = inputs["wq"]
    wkv = inputs["wkv"]
    wqp = inputs["wqp"]
    wkvp = inputs["wkvp"]
    wg = inputs["wg"]
    gw = np.asarray(inputs["geom_weight"], np.float32)
    hw = np.asarray(inputs["head_weights"], np.float32)
    sh = 1.0 / (1.0 + np.exp(-hw))

    wall = np.zeros((CS, WALL_COLS), np.float32)
    wall[:, K_OFF:K_OFF + 192] = wkv[:, :192]
    wall[:, V_OFF:V_OFF + 192] = wkv[:, 192:]
    wall[:, KVP_OFF:KVP_OFF + 864] = wkvp
    wall[:, Q_OFF:Q_OFF + 192] = wq
    wall[:, G_OFF:G_OFF + 48] = wg
    wall[:, QP_OFF:QP_OFF + 288] = wqp

    bout_half = np.asarray(inputs["bout"], np.float32)[None, :] * 0.5
    wout_b = np.concatenate(
        [np.asarray(inputs["wout"], np.float32), bout_half, bout_half],
        axis=0)

    # qf/kf feature layout per head (FEAT=64 stride):
    #  [0:16] scalar q/k; [16+8i : 20+8i] coords_i (4 pts); [20+8i : 24+8i]
    #  dirs_i; [40] k2 / const; [41] q-combo; rest pad
    qs = np.zeros((FEAT * H,), np.float32)
    for h in range(H):
        o = h * FEAT
        qs[o:o + 16] = sh[h] / np.sqrt(C)
        for i in range(3):
            qs[o + 16 + 8 * i:o + 20 + 8 * i] = sh[h] * gw[0] * 0.5
            qs[o + 20 + 8 * i:o + 24 + 8 * i] = sh[h] * gw[1]
        qs[o + 40] = -sh[h] * gw[0] / P
        qs[o + 41] = 1.0
    qscale = np.broadcast_to(qs, (128, FEAT * H)).copy()

    rot9 = np.ascontiguousarray(
        np.asarray(inputs["rot"], np.float32).reshape(B, N, 9))
    trans = np.asarray(inputs["trans"], np.float32)
    s = np.asarray(inputs["s"], np.float32)

    def bfbits(a):
        u = np.ascontiguousarray(a, np.float32).view(np.uint32)
        r = ((u >> 16) + ((u >> 15) & 1)).astype(np.uint32)
        return (r & 0xFFFF).astype(np.uint16)

    sT = np.ascontiguousarray(s.transpose(0, 2, 1))      # [B, CS, N]
    return (s, sT, rot9, trans, bfbits(wall), bfbits(wout_b), qscale,
            bfbits, gw)


_PROGRAM_CACHE = {}


def _build_program(gw0, gw1):
    key = (float(gw0), float(gw1))
    if key in _PROGRAM_CACHE:
        return _PROGRAM_CACHE[key]

    nc = bacc.Bacc("TRN2", target_bir_lowering=False, debug=False, num_devices=8)

    sT_d = nc.dram_tensor("sT_d", [CS, N], BF16, kind="ExternalInput")
    qsT_d = nc.dram_tensor("qsT_d", [CS, NB], BF16, kind="ExternalInput")
    rot_full = nc.dram_tensor("rot_full", [N, 9], F32, kind="ExternalInput")
    trans_full = nc.dram_tensor("trans_full", [N, 3], F32, kind="ExternalInput")
    q_rot = nc.dram_tensor("q_rot", [NB, 9], F32, kind="ExternalInput")
    q_trans = nc.dram_tensor("q_trans", [NB, 3], F32, kind="ExternalInput")
    wall_d = nc.dram_tensor("wall", [CS, WALL_COLS], BF16, kind="ExternalInput")
    wout_d = nc.dram_tensor("wout_b", [FOUT + 2, CS], BF16, kind="ExternalInput")
    qscale_d = nc.dram_tensor("qscale", [128, FEAT * H], F32, kind="ExternalInput")
    out_loc = nc.dram_tensor("out_loc", [NB, CS], F32, kind="ExternalOutput")

    with tile.TileContext(nc) as tc:
        with ExitStack() as ctx:
            _emit(ctx, tc, nc, sT_d, qsT_d, rot_full, trans_full, q_rot,
                  q_trans, wall_d, wout_d, qscale_d, out_loc, gw0, gw1)

    nc.compile()
    _PROGRAM_CACHE[key] = nc
    return nc


def _emit(ctx, tc, nc, sT_d, qsT_d, rot_full, trans_full, q_rot, q_trans,
          wall_d, wout_d, qscale_d, out_loc, gw0, gw1):
    PS = bass.MemorySpace.PSUM

    const = ctx.enter_context(tc.tile_pool(name="const", bufs=1))
    work = ctx.enter_context(tc.tile_pool(name="work", bufs=1))
    attA = ctx.enter_context(tc.tile_pool(name="attA", bufs=1))
    tmp_pool = ctx.enter_context(tc.tile_pool(name="tmp", bufs=2))
    pA_ctx = ExitStack()
    pA = pA_ctx.enter_context(tc.tile_pool(name="pA", bufs=1))
    pre_ctx = ExitStack()
    tpsum = pre_ctx.enter_context(tc.tile_pool(name="tpsum", bufs=2, space=PS))

    # ---- input DMA (consumption order) -----------------------------------
    qsT = [pA.tile([128, NB], BF16, name=f"qsT{kc}") for kc in range(3)]
    for kc in range(3):
        nc.sync.dma_start(qsT[kc][:], qsT_d[kc * 128:(kc + 1) * 128, :])
    wall_sb = []
    for kc in range(3):
        t = pA.tile([128, WALL_COLS], BF16, name=f"wall{kc}")
        nc.sync.dma_start(t[:], wall_d[kc * 128:(kc + 1) * 128, :])
        wall_sb.append(t)
    sT = [pA.tile([128, N], BF16, name=f"sT{kc}") for kc in range(3)]
    for kc in range(3):
        nc.sync.dma_start(sT[kc][:], sT_d[kc * 128:(kc + 1) * 128, :])
    rot_all = const.tile([128, KT * 9], F32, name="rot_all")
    nc.sync.dma_start(rot_all[:].rearrange("p (n c) -> p n c", c=9),
                      rot_full[:].rearrange("(n p) c -> p n c", p=128))
    rot_k = [rot_all[:, kt * 9:(kt + 1) * 9] for kt in range(KT)]
    trn_all = const.tile([128, KT * 3], F32, name="trn_all")
    nc.sync.dma_start(trn_all[:].rearrange("p (n c) -> p n c", c=3),
                      trans_full[:].rearrange("(n p) c -> p n c", p=128))
    qrot_all = const.tile([128, NT * 9], F32, name="qrot_all")
    nc.sync.dma_start(qrot_all[:].rearrange("p (n c) -> p n c", c=9),
                      q_rot[:].rearrange("(n p) c -> p n c", p=128))
    rot_sb = [qrot_all[:, nt * 9:(nt + 1) * 9] for nt in range(NT)]
    qtr_all = const.tile([128, NT * 3], F32, name="qtr_all")
    nc.sync.dma_start(qtr_all[:].rearrange("p (n c) -> p n c", c=3),
                      q_trans[:].rearrange("(n p) c -> p n c", p=128))
    trans_sb = [qtr_all[:, nt * 3:(nt + 1) * 3] for nt in range(NT)]
    qscale_sb = const.tile([128, FEAT * H], F32)
    nc.sync.dma_start(qscale_sb[:], qscale_d[:, :])

    # ---- constants -------------------------------------------------------
    rot_bf = const.tile([128, KT * 9], BF16, name="rot_bf")
    nc.vector.tensor_copy(rot_bf[:], rot_all[:])
    ident = const.tile([128, 128], F32)
    make_identity(nc, ident[:])
    ident_r = const.tile([128, 128], F32R)
    nc.vector.tensor_copy(ident_r[:], ident[:])
    ones2_f32 = const.tile([2, NB], F32)
    nc.gpsimd.memset(ones2_f32[:], 1.0)

    # ---- K/V-side projections (8 key tiles, cols 0:1248) -----------------
    kvp_sb = [pA.tile([128, 864], BF16, name=f"kvp{kt}") for kt in range(KT)]
    kf = [pA.tile([128, FEAT * H], F32R, name=f"kf{kt}") for kt in range(KT)]
    kfG = [attA.tile([128, N], BF16, name=f"kfG{t}") for t in range(6)]
    va = [attA.tile([128, OCH * H], BF16, name=f"va{kt}") for kt in range(KT)]
    ppsum = pre_ctx.enter_context(tc.tile_pool(name="ppsum", bufs=2, space=PS))

    def proj_block(sTl, nsl, c0, c1, outs):
        ps = ppsum.tile([128, c1 - c0], F32, tag="proj", name="ps")
        for kc in range(3):
            nc.tensor.matmul(ps[:], sTl[kc][:, nsl], wall_sb[kc][:, c0:c1],
                             start=(kc == 0), stop=(kc == 2))
        for (lo, hi, dv, kind) in outs:
            src = ps[:, lo - c0:hi - c0]
            if kind == "copy":
                nc.vector.tensor_copy(dv, src)
            elif kind == "sigmoid":
                nc.scalar.activation(dv, src, AF.Sigmoid)
            else:
                nc.scalar.activation(dv, src, AF.Relu)

    # ---- paired rigid transform ------------------------------------------
    # src layout per point: [c0 d0 c1 d1 c2 d2]; pair j = cols 2j:2j+2.
    # out pair i gets sum_j pv[:, :, :, j, :] * rt[j, i]; coords then -= 0
    # / += trans via a second pass on the coord half only.
    def emit_tf(cosrc, disrc, codst, didst, rt, tr, s1=False):
        """cosrc(j)/disrc(j): [128, H, x] views; codst/didst[i] same shape.
        s1=True puts the first madd of each output on the scalar engine."""
        for i in range(3):
            dv = codst[i]
            nc.vector.tensor_scalar(dv, cosrc(0), rt[:, 3 * i:3 * i + 1],
                                    tr[:, i:i + 1], OP.mult, OP.add)
            nc.vector.scalar_tensor_tensor(dv, cosrc(1),
                                           rt[:, 3 * i + 1:3 * i + 2], dv,
                                           OP.mult, OP.add)
            nc.vector.scalar_tensor_tensor(dv, cosrc(2),
                                           rt[:, 3 * i + 2:3 * i + 3], dv,
                                           OP.mult, OP.add)
            dv = didst[i]
            if s1:
                nc.scalar.activation(dv, disrc(0), AF.Copy,
                                     scale=rt[:, 3 * i:3 * i + 1])
            else:
                nc.vector.tensor_scalar_mul(dv, disrc(0),
                                            rt[:, 3 * i:3 * i + 1])
            nc.vector.scalar_tensor_tensor(dv, disrc(1),
                                           rt[:, 3 * i + 1:3 * i + 2], dv,
                                           OP.mult, OP.add)
            nc.vector.scalar_tensor_tensor(dv, disrc(2),
                                           rt[:, 3 * i + 2:3 * i + 3], dv,
                                           OP.mult, OP.add)

    def sumsq(eng, dst, t2, cviews):
        eng.tensor_tensor(dst[:], cviews[0], cviews[0], OP.mult)
        for cv in cviews[1:]:
            eng.tensor_tensor(t2[:], cv, cv, OP.mult)
            eng.tensor_tensor(dst[:], dst[:], t2[:], OP.add)

    def psum4(eng, dst, srct):
        sv = srct[:].rearrange("p (h x) -> p h x", x=P)
        eng.tensor_tensor(dst, sv[:, :, 0], sv[:, :, 1], OP.add)
        eng.tensor_tensor(dst, dst, sv[:, :, 2], OP.add)
        eng.tensor_tensor(dst, dst, sv[:, :, 3], OP.add)

    expT_tiles = [work.tile([128, 2048], BF16, name=f"expT{i}")
                  for i in range(H)]
    qpsum = pre_ctx.enter_context(tc.tile_pool(name="qpsum", bufs=2, space=PS))

    # ---- Q-side ----------------------------------------------------------
    g_sb = [work.tile([128, 48], F32, name=f"g{nt}") for nt in range(NT)]
    qp_sb = [work.tile([128, 288], BF16, name=f"qp{nt}") for nt in range(NT)]
    qkvp_sb = [work.tile([128, 864], BF16, name=f"qkvp{nt}") for nt in range(NT)]
    qf = [work.tile([128, FEAT * H], F32, name=f"qf{nt}") for nt in range(NT)]
    qsv = qscale_sb[:].rearrange("p (h f) -> p h f", f=FEAT)
    for nt in range(NT):
        nsl = slice(nt * 128, (nt + 1) * 128)
        qfv = qf[nt][:].rearrange("p (h f) -> p h f", f=FEAT)
        proj_block(qsT, nsl, Q_OFF, Q_OFF + 240,
                   [(Q_OFF, Q_OFF + 192, qfv[:, :, 0:16], "copy"),
                    (G_OFF, G_OFF + 48, g_sb[nt][:], "sigmoid")])
        proj_block(qsT, nsl, QP_OFF, QP_OFF + 288,
                   [(QP_OFF, QP_OFF + 288, qp_sb[nt][:], "relu")])
        proj_block(qsT, nsl, KVP_OFF, KVP_OFF + 432,
                   [(KVP_OFF, KVP_OFF + 432, qkvp_sb[nt][:, 0:432], "relu")])
        proj_block(qsT, nsl, KVP_OFF + 432, KVP_OFF + 864,
                   [(KVP_OFF + 432, KVP_OFF + 864, qkvp_sb[nt][:, 432:864],
                     "relu")])

    for nt in range(NT):
        qfv = qf[nt][:].rearrange("p (h f) -> p h f", f=FEAT)
        qpv = qp_sb[nt][:].rearrange("p (h x c) -> p h x c", x=4, c=6)
        qf_coords = [qfv[:, :, 16 + 8 * i:20 + 8 * i] for i in range(3)]
        qf_dirs = [qfv[:, :, 20 + 8 * i:24 + 8 * i] for i in range(3)]
        emit_tf(lambda j: qpv[:, :, :, j], lambda j: qpv[:, :, :, 3 + j],
                qf_coords, qf_dirs, rot_sb[nt], trans_sb[nt])
        # gate all 6 comps (coords+dirs)
        gv = g_sb[nt][:].rearrange("p (h x) -> p h x", x=P)
        for i in range(6):
            half = qfv[:, :, 16 + 4 * i:20 + 4 * i]
            nc.vector.tensor_tensor(half, half, gv, OP.mult)

    # k-dirs for q rows (curvature), dirs only
    pco_qk = [work.tile([128, 3 * 48], F32, name=f"pqk{nt}") for nt in range(NT)]
    for nt in range(NT):
        pv = qkvp_sb[nt][:].rearrange("p (h x c) -> p h x c", x=12, c=6)
        rt = rot_sb[nt]
        for i in range(3):
            ddi = pco_qk[nt][:, i * 48:(i + 1) * 48].rearrange(
                "p (h x) -> p h x", x=4)
            nc.vector.tensor_scalar_mul(ddi, pv[:, :, 0:4, 3],
                                        rt[:, 3 * i:3 * i + 1])
            nc.vector.scalar_tensor_tensor(ddi, pv[:, :, 0:4, 4],
                                           rt[:, 3 * i + 1:3 * i + 2], ddi,
                                           OP.mult, OP.add)
            nc.vector.scalar_tensor_tensor(ddi, pv[:, :, 0:4, 5],
                                           rt[:, 3 * i + 2:3 * i + 3], ddi,
                                           OP.mult, OP.add)

    def comp_qk(nt, i):
        return pco_qk[nt][:, i * 48:(i + 1) * 48].rearrange(
            "p (h x) -> p h x", x=4)

    # ---- q-side reductions into qf cols 40/41 ----------------------------
    for nt in range(NT):
        qfv = qf[nt][:].rearrange("p (h f) -> p h f", f=FEAT)
        qf_coords = [qfv[:, :, 16 + 8 * i:20 + 8 * i] for i in range(3)]
        qf_dirs = [qfv[:, :, 20 + 8 * i:24 + 8 * i] for i in range(3)]
        eng = nc.gpsimd if nt == 0 else nc.vector
        sq = tmp_pool.tile([128, 48], F32, tag="sq", name="sq")
        t2 = tmp_pool.tile([128, 48], F32, tag="t2", name="t2")
        sumsq(eng, sq, t2, qf_coords)
        psum4(eng, qfv[:, :, 41], sq)

        cr = tmp_pool.tile([128, 48], F32, tag="cr", name="cr")
        cs_ = tmp_pool.tile([128, 48], F32, tag="cs_", name="cs_")
        t3 = tmp_pool.tile([128, 48], F32, tag="t3", name="t3")
        first = True
        for (a, b_) in ((1, 2), (2, 0), (0, 1)):
            eng.tensor_tensor(cr[:], qf_dirs[a], comp_qk(nt, b_), OP.mult)
            eng.tensor_tensor(t3[:], qf_dirs[b_], comp_qk(nt, a), OP.mult)
            eng.tensor_tensor(cr[:], cr[:], t3[:], OP.subtract)
            eng.tensor_tensor(cr[:], cr[:], cr[:], OP.mult)
            if first:
                eng.tensor_copy(cs_[:], cr[:])
                first = False
            else:
                eng.tensor_tensor(cs_[:], cs_[:], cr[:], OP.add)
        nq2 = tmp_pool.tile([128, 48], F32, tag="nq2", name="nq2")
        nk2 = tmp_pool.tile([128, 48], F32, tag="nk2", name="nk2")
        sumsq(eng, nq2, t2, qf_dirs)
        sumsq(eng, nk2, t2, [comp_qk(nt, i) for i in range(3)])
        eng.tensor_tensor(nq2[:], nq2[:], nk2[:], OP.mult)
        nc.scalar.activation(nq2[:], nq2[:], AF.Sqrt)
        nc.vector.tensor_scalar_add(nq2[:], nq2[:], EPS)
        nc.vector.reciprocal(nq2[:], nq2[:])
        nc.scalar.activation(cs_[:], cs_[:], AF.Sqrt,
                             scale=float((gw1 / gw0) ** 2))
        eng.tensor_tensor(cs_[:], cs_[:], nq2[:], OP.mult)
        csv = cs_[:].rearrange("p (h x) -> p h x", x=P)
        for x in range(P):
            eng.tensor_tensor(qfv[:, :, 41], qfv[:, :, 41],
                              csv[:, :, x], OP.add)
        eng.tensor_tensor(qfv[:, :, 41], qfv[:, :, 41],
                          qsv[:, :, 40], OP.mult)
        eng.tensor_copy(qfv[:, :, 40], qsv[:, :, 40])

    # warm the EXP table off the critical path
    warm = tmp_pool.tile([2, 2], F32, tag="warm", name="warm")
    nc.vector.memset(warm[:], 0.0)
    nc.scalar.activation(warm[:], warm[:], AF.Exp)

    for nt in range(NT):
        qfv = qf[nt][:].rearrange("p (h f) -> p h f", f=FEAT)
        nc.vector.tensor_tensor(qfv[:, :, 0:40], qfv[:, :, 0:40],
                                qsv[:, :, 0:40], OP.mult)

    qfT = [work.tile([128, NB], BF16, name=f"qfT{t}") for t in range(6)]
    for t in range(6):
        for nt in range(NT):
            ps = tpsum.tile([128, 128], F32, tag="tps")
            nc.tensor.transpose(ps[:], qf[nt][:, t * 128:(t + 1) * 128], ident[:])
            if t % 2:
                nc.scalar.copy(qfT[t][:, nt * 128:(nt + 1) * 128], ps[:])
            else:
                nc.vector.tensor_copy(qfT[t][:, nt * 128:(nt + 1) * 128], ps[:])

    for kt in range(KT):
        ksl = slice(kt * 128, (kt + 1) * 128)
        kfv = kf[kt][:].rearrange("p (h f) -> p h f", f=FEAT)
        vav = va[kt][:].rearrange("p (h f) -> p h f", f=OCH)
        proj_block(sT, ksl, 0, 512,
                   [(0, 192, kfv[:, :, 0:16], "copy"),
                    (192, 384, vav[:, :, 0:16], "copy"),
                    (384, 512, kvp_sb[kt][:, 0:128], "relu")])
        proj_block(sT, ksl, 512, 1024,
                   [(512, 1024, kvp_sb[kt][:, 128:640], "relu")])
        proj_block(sT, ksl, 1024, 1248,
                   [(1024, 1248, kvp_sb[kt][:, 640:864], "relu")])

    for kt in range(KT):
        kfv = kf[kt][:].rearrange("p (h f) -> p h f", f=FEAT)
        vav = va[kt][:].rearrange("p (h f) -> p h f", f=OCH)
        pv_h = kvp_sb[kt][:].rearrange("p (h x c) -> p h x c", x=12, c=6)
        kf_coords = [kfv[:, :, 16 + 8 * i:20 + 8 * i] for i in range(3)]
        kf_dirs = [kfv[:, :, 20 + 8 * i:24 + 8 * i] for i in range(3)]
        emit_tf(lambda j: pv_h[:, :, 0:4, j], lambda j: pv_h[:, :, 0:4, 3 + j],
                kf_coords, kf_dirs, rot_k[kt],
                trn_all[:, kt * 3:(kt + 1) * 3])
        va_coords = [vav[:, :, 16 + 16 * i:24 + 16 * i] for i in range(3)]
        va_dirs = [vav[:, :, 24 + 16 * i:32 + 16 * i] for i in range(3)]
        rt = rot_k[kt]
        tr = trn_all[:, kt * 3:(kt + 1) * 3]
        for i in range(3):
            dv = va_coords[i]
            nc.vector.tensor_scalar(dv, pv_h[:, :, 4:12, 0],
                                    rt[:, 3 * i:3 * i + 1],
                                    tr[:, i:i + 1], OP.mult, OP.add)
            nc.vector.scalar_tensor_tensor(dv, pv_h[:, :, 4:12, 1],
                                           rt[:, 3 * i + 1:3 * i + 2], dv,
                                           OP.mult, OP.add)
            nc.vector.scalar_tensor_tensor(dv, pv_h[:, :, 4:12, 2],
                                           rt[:, 3 * i + 2:3 * i + 3], dv,
                                           OP.mult, OP.add)

        def rbc(c):  # [128, H, 8] stride-0 broadcast of rot_bf col
            base = rot_bf[:, kt * 9 + c:kt * 9 + c + 1]
            return bass.AP(base.tensor, base.offset,
                           [tuple(base.ap[0]), (0, H), (0, 8)])

        for i in range(3):
            dv = va_dirs[i]
            tmpb = tmp_pool.tile([128, 96], BF16, tag="tfb", name="tfb")
            tv = tmpb[:].rearrange("p (h x) -> p h x", x=8)
            nc.gpsimd.tensor_tensor(dv, pv_h[:, :, 4:12, 3],
                                    rbc(3 * i), OP.mult)
            nc.gpsimd.tensor_tensor(tv, pv_h[:, :, 4:12, 4],
                                    rbc(3 * i + 1), OP.mult)
            nc.gpsimd.tensor_tensor(dv, dv, tv, OP.add)
            nc.gpsimd.tensor_tensor(tv, pv_h[:, :, 4:12, 5],
                                    rbc(3 * i + 2), OP.mult)
            nc.gpsimd.tensor_tensor(dv, dv, tv, OP.add)
        # k2: sum of squared coords over pts and comps
        sq = tmp_pool.tile([128, 48], F32, tag="sq", name="sq")
        t2 = tmp_pool.tile([128, 48], F32, tag="t2", name="t2")
        sumsq(nc.gpsimd, sq, t2, kf_coords)
        psum4(nc.gpsimd, kfv[:, :, 40], sq)
        nc.gpsimd.memset(kfv[:, :, 41].bitcast(F32), 1.0)
        nc.gpsimd.memset(vav[:, :, 64], 1.0)
        nc.gpsimd.memset(vav[:, :, 65], 0.0)
        for t in range(6):
            ps = tpsum.tile([128, 128], F32R, tag="tpsr")
            nc.tensor.transpose(ps[:], kf[kt][:, t * 128:(t + 1) * 128],
                                ident_r[:])
            if (kt + t) % 2:
                nc.scalar.copy(kfG[t][:, kt * 128:(kt + 1) * 128],
                               ps[:].bitcast(F32))
            else:
                nc.vector.tensor_copy(kfG[t][:, kt * 128:(kt + 1) * 128],
                                      ps[:].bitcast(F32))
        # fused QK + EXP for this key tile, all heads
        for h in range(H):
            t, base = h // 2, (h % 2) * 64
            aps = qpsum.tile([128, NB], F32, tag="qk", name="aps")
            nc.tensor.matmul(aps[:],
                             kfG[t][base:base + FS, kt * 128:(kt + 1) * 128],
                             qfT[t][base:base + FS, :],
                             start=True, stop=True)
            nc.scalar.activation(
                expT_tiles[h][:, kt * NB:(kt + 1) * NB], aps[:], AF.Exp)

    # ---- inverse transform helper ----------------------------------------
    feats = [work.tile([128, FOUT], F32, name=f"feats{qt}") for qt in range(NT)]
    tinv = [work.tile([128, 3], F32, name=f"tinv{qt}") for qt in range(NT)]
    for qt in range(NT):
        rt, tr = rot_sb[qt], trans_sb[qt]
        for i in range(3):
            nc.vector.tensor_scalar_mul(tinv[qt][:, i:i + 1], tr[:, 0:1],
                                        rt[:, i:i + 1])
            nc.vector.scalar_tensor_tensor(tinv[qt][:, i:i + 1], tr[:, 1:2],
                                           rt[:, 3 + i:4 + i], tinv[qt][:, i:i + 1],
                                           OP.mult, OP.add)
            nc.vector.scalar_tensor_tensor(tinv[qt][:, i:i + 1], tr[:, 2:3],
                                           rt[:, 6 + i:7 + i], tinv[qt][:, i:i + 1],
                                           OP.mult, OP.add)

    def emit_inverse(qt, hh):
        hs = slice(hh * 6, hh * 6 + 6)
        ovv = o_all[qt][:].rearrange("p (h f) -> p h f", f=FEAT)[:, hs]

        def ogp(j):
            return ovv[:, :, 16 + 16 * j:32 + 16 * j]

        nc.vector.tensor_copy(
            feats[qt][:, hh * 96:hh * 96 + 96].rearrange("p (h c) -> p h c", c=16),
            ovv[:, :, 0:16])
        gview = feats[qt][:, 192:FOUT].rearrange(
            "p (h x c) -> p h x c", h=H, c=7)[:, hs]
        rt = rot_sb[qt]

        lcld = [tmp_pool.tile([128, 96], F32, tag=f"lcld{i}", name=f"lcld{i}")
                for i in range(3)]
        for i in range(3):
            lv = lcld[i][:].rearrange("p (h x) -> p h x", x=16)
            nc.vector.tensor_scalar_mul(lv, ogp(0), rt[:, i:i + 1])
            nc.vector.scalar_tensor_tensor(lv, ogp(1), rt[:, 3 + i:4 + i],
                                           lv, OP.mult, OP.add)
            nc.vector.scalar_tensor_tensor(lv, ogp(2), rt[:, 6 + i:7 + i],
                                           lv, OP.mult, OP.add)
            nc.vector.tensor_scalar(lv[:, :, 0:8], lv[:, :, 0:8],
                                    tinv[qt][:, i:i + 1], None, OP.subtract)
        n2 = tmp_pool.tile([128, 96], F32, tag="n2", name="n2")
        t2b = tmp_pool.tile([128, 96], F32, tag="t2b", name="t2b")
        nc.gpsimd.tensor_tensor(n2[:], lcld[0][:], lcld[0][:], OP.mult)
        for i in (1, 2):
            nc.gpsimd.tensor_tensor(t2b[:], lcld[i][:], lcld[i][:], OP.mult)
            nc.gpsimd.tensor_tensor(n2[:], n2[:], t2b[:], OP.add)
        n2v = n2[:].rearrange("p (h x) -> p h x", x=16)
        nc.scalar.activation(gview[:, :, :, 6], n2v[:, :, 0:8], AF.Sqrt)
        nc.scalar.activation(n2v[:, :, 8:16], n2v[:, :, 8:16], AF.Sqrt)
        ndv = n2v[:, :, 8:16]
        nc.vector.tensor_scalar_max(ndv, ndv, EPS)
        nc.vector.reciprocal(ndv, ndv)
        for i in range(3):
            lv = lcld[i][:].rearrange("p (h x) -> p h x", x=16)
            nc.gpsimd.tensor_copy(gview[:, :, :, i], lv[:, :, 0:8])
            nc.gpsimd.tensor_tensor(gview[:, :, :, 3 + i],
                                    lv[:, :, 8:16], ndv, OP.mult)

    # ---- attention AV (QK/EXP ran fused in the K-side loop) --------------
    pre_ctx.close()
    pA_ctx.close()
    att_ctx = ExitStack()
    opsum = att_ctx.enter_context(tc.tile_pool(name="opsum", bufs=2, space=PS))
    o_all = [work.tile([128, FEAT * H], F32, name=f"oall{qt}") for qt in range(NT)]

    def emit_av(h):
        expT = expT_tiles[h]
        ot_ps = opsum.tile([OCH, NB], F32, tag="otacc", name="ot_ps")
        for kb in range(NKB):
            nc.tensor.matmul(
                ot_ps[:],
                va[kb][:, h * OCH:(h + 1) * OCH],
                expT[:, kb * NB:(kb + 1) * NB],
                start=(kb == 0), stop=(kb == NKB - 1))
        ot_sb = tmp_pool.tile([OCH, NB], F32R, tag="otsb", name="otsb", bufs=2)
        nc.vector.tensor_copy(ot_sb[:], ot_ps[:])
        for qt in range(NT):
            tp = opsum.tile([128, OCH], F32R, tag="otp", name="tp")
            nc.tensor.transpose(tp[:], ot_sb[:, qt * 128:(qt + 1) * 128],
                                ident_r[0:OCH, 0:OCH])
            rec = tmp_pool.tile([128, 1], F32, tag="rec", name="rec")
            nc.vector.reciprocal(rec[:], tp[:, 64:65].bitcast(F32))
            nc.vector.tensor_scalar_mul(
                o_all[qt][:, h * FEAT:h * FEAT + 64], tp[:, 0:64].bitcast(F32),
                rec[:])

    for h in range(H):
        emit_av(h)
        if h == 6:
            for qt in range(NT):
                emit_inverse(qt, 0)
    for qt in range(NT):
        emit_inverse(qt, 1)

    # ---- output projection -----------------------------------------------
    att_ctx.close()
    wout_sb = []
    for kc in range(KCH):
        r0 = kc * 128
        r1 = min(FOUT + 2, r0 + 128)
        t = const.tile([r1 - r0, CS], BF16, name=f"wout{kc}")
        nc.sync.dma_start(t[:], wout_d[r0:r1, :])
        wout_sb.append(t)
    tpsum2 = ctx.enter_context(tc.tile_pool(name="tpsum2", bufs=2, space=PS))
    opsum2 = ctx.enter_context(tc.tile_pool(name="opsum2", bufs=2, space=PS))
    fT = []
    for kc in range(KCH):
        r0 = kc * 128
        rw = min(FOUT, r0 + 128) - r0
        pw = rw + 2 if kc == KCH - 1 else rw
        t = work.tile([pw, NB], BF16, name=f"fT{kc}")
        fT.append(t)
    lastr = FOUT - (KCH - 1) * 128
    nc.vector.tensor_copy(fT[KCH - 1][lastr:lastr + 2, :], ones2_f32[:])
    for kc in range(KCH):
        r0 = kc * 128
        rw = min(FOUT, r0 + 128) - r0
        for qt in range(NT):
            ps = tpsum2.tile([128, 128], F32, tag="tps2")
            nc.tensor.transpose(ps[:rw, :], feats[qt][:, r0:r0 + rw], ident[:])
            if kc % 2:
                nc.scalar.copy(fT[kc][:rw, qt * 128:(qt + 1) * 128], ps[:rw, :])
            else:
                nc.vector.tensor_copy(fT[kc][:rw, qt * 128:(qt + 1) * 128],
                                      ps[:rw, :])

    for qt in range(NT):
        ps = opsum2.tile([128, CS], F32, tag="oproj")
        for kc in range(KCH):
            nc.tensor.matmul(ps[:], fT[kc][:, qt * 128:(qt + 1) * 128],
                             wout_sb[kc][:], start=(kc == 0), stop=(kc == KCH - 1))
        osb = tmp_pool.tile([128, CS], F32, tag="osb", name="osb")
        nc.scalar.copy(osb[:], ps[:])
        nc.sync.dma_start(out_loc[qt * 128:(qt + 1) * 128, :], osb[:])


def _run(inputs, trace=False):
    (s, sT, rot9, trans, wall, wout_b, qscale, bfbits, gw) = _host_prep(inputs)
    nc = _build_program(float(gw[0]), float(gw[1]))
    in_maps = []
    for c in range(8):
        b, qb = c // 4, c % 4
        r = slice(qb * NB, (qb + 1) * NB)
        in_maps.append({
            "sT_d": bfbits(sT[b]),
            "qsT_d": bfbits(sT[b][:, r]),
            "rot_full": np.ascontiguousarray(rot9[b]),
            "trans_full": np.ascontiguousarray(trans[b]),
            "q_rot": np.ascontiguousarray(rot9[b, r]),
            "q_trans": np.ascontiguousarray(trans[b, r]),
            "wall": wall, "wout_b": wout_b, "qscale": qscale,
        })
    res = run_bass_kernel_spmd(nc, in_maps, list(range(8)), trace=trace)
    out = np.empty((B, N, CS), np.float32)
    for c in range(8):
        b, qb = c // 4, c % 4
        out[b, qb * NB:(qb + 1) * NB] = res.results[c]["out_loc"]
    return out, res


def kernel(**inputs):
    out, _ = _run(inputs, trace=False)
    return out


def kernel_traced(**inputs):
    return _run(inputs, trace=True)


# revision 49
# speedup vs baseline: 1.0241x; 1.0241x over previous
"""Bass/Tile TRN2 kernel for EnhancedIPA3 (invariant-point-attention variant).

Sharding: 8 cores = batch(2) x query-block(4).  NO collectives: each core
recomputes the full K/V side for its batch locally (deterministic; the
AllGather variant was hostage to inter-core launch skew).  Query side runs
on the core's own 256 rows (separately-sliced inputs).

Point channels are pair-interleaved [c0 d0 c1 d1 c2 d2] per point so each
rigid-transform madd covers coord+dir at double width, and the transforms
write straight into the attention feature tiles (kf/va/qf) -- no staging
buffers or assembly copies.  s arrives host-pretransposed in bf16.  All
biases are zero per the problem spec and are dropped.

Self-contained: hardcodes all shapes; only depends on numpy + concourse.
"""

import numpy as np
from contextlib import ExitStack

import concourse.bass as bass
import concourse.bacc as bacc
import concourse.mybir as mybir
import concourse.tile as tile
from concourse.bass_utils import run_bass_kernel_spmd
from concourse.masks import make_identity

F32 = mybir.dt.float32
F32R = mybir.dt.float32r
BF16 = mybir.dt.bfloat16
AF = mybir.ActivationFunctionType
OP = mybir.AluOpType

B, N, CS, H, C, P, V = 2, 1024, 384, 12, 16, 4, 8
EPS = 1e-8
NB = N // 4            # 256 query rows per core
NT = NB // 128
KT = N // 128          # 8 key row-tiles (full batch, local)
K_OFF, V_OFF, KVP_OFF = 0, 192, 384
Q_OFF, G_OFF, QP_OFF = 1248, 1440, 1488
WALL_COLS = 1776
FEAT = 64              # per-head feature stride in kf/qf
FS = 42
OCH = 66
FOUT = H * (C + 7 * V)
KCH = 7
NKB = 8


def _host_prep(inputs):
    wq = inputs["wq"]
    wkv = inputs["wkv"]
    wqp = inputs["wqp"]
    wkvp = inputs["wkvp"]
    wg = inputs["wg"]
    gw = np.asarray(inputs["geom_weight"], np.float32)
    hw = np.asarray(inputs["head_weights"], np.float32)
    sh = 1.0 / (1.0 + np.exp(-hw))

    wall = np.zeros((CS, WALL_COLS), np.float32)
    wall[:, K_OFF:K_OFF + 192] = wkv[:, :192]
    wall[:, V_OFF:V_OFF + 192] = wkv[:, 192:]
    wall[:, KVP_OFF:KVP_OFF + 864] = wkvp
    wall[:, Q_OFF:Q_OFF + 192] = wq
    wall[:, G_OFF:G_OFF + 48] = wg
    wall[:, QP_OFF:QP_OFF + 288] = wqp

    bout_half = np.asarray(inputs["bout"], np.float32)[None, :] * 0.5
    wout_b = np.concatenate(
        [np.asarray(inputs["wout"], np.float32), bout_half, bout_half],
        axis=0)

    # qf/kf feature layout per head (FEAT=64 stride):
    #  [0:16] scalar q/k; [16+8i : 20+8i] coords_i (4 pts); [20+8i : 24+8i]
    #  dirs_i; [40] k2 / const; [41] q-combo; rest pad
    qs = np.zeros((FEAT * H,), np.float32)
    for h in range(H):
        o = h * FEAT
        qs[o:o + 16] = sh[h] / np.sqrt(C)
        for i in range(3):
            qs[o + 16 + 8 * i:o + 20 + 8 * i] = sh[h] * gw[0] * 0.5
            qs[o + 20 + 8 * i:o + 24 + 8 * i] = sh[h] * gw[1]
        qs[o + 40] = -sh[h] * gw[0] / P
        qs[o + 41] = 1.0
    qscale = np.broadcast_to(qs, (128, FEAT * H)).copy()

    rot9 = np.ascontiguousarray(
        np.asarray(inputs["rot"], np.float32).reshape(B, N, 9))
    trans = np.asarray(inputs["trans"], np.float32)
    s = np.asarray(inputs["s"], np.float32)

    def bfbits(a):
        u = np.ascontiguousarray(a, np.float32).view(np.uint32)
        r = ((u >> 16) + ((u >> 15) & 1)).astype(np.uint32)
        return (r & 0xFFFF).astype(np.uint16)

    sT = np.ascontiguousarray(s.transpose(0, 2, 1))      # [B, CS, N]
    return (s, sT, rot9, trans, bfbits(wall), bfbits(wout_b), qscale,
            bfbits, gw)


_PROGRAM_CACHE = {}


def _build_program(gw0, gw1):
    key = (float(gw0), float(gw1))
    if key in _PROGRAM_CACHE:
        return _PROGRAM_CACHE[key]

    nc = bacc.Bacc("TRN2", target_bir_lowering=False, debug=False, num_devices=8)

    sT_d = nc.dram_tensor("sT_d", [CS, N], BF16, kind="ExternalInput")
    qsT_d = nc.dram_tensor("qsT_d", [CS, NB], BF16, kind="ExternalInput")
    rot_full = nc.dram_tensor("rot_full", [N, 9], F32, kind="ExternalInput")
    trans_full = nc.dram_tensor("trans_full", [N, 3], F32, kind="ExternalInput")
    q_rot = nc.dram_tensor("q_rot", [NB, 9], F32, kind="ExternalInput")
    q_trans = nc.dram_tensor("q_trans", [NB, 3], F32, kind="ExternalInput")
    wall_d = nc.dram_tensor("wall", [CS, WALL_COLS], BF16, kind="ExternalInput")
    wout_d = nc.dram_tensor("wout_b", [FOUT + 2, CS], BF16, kind="ExternalInput")
    qscale_d = nc.dram_tensor("qscale", [128, FEAT * H], F32, kind="ExternalInput")
    out_loc = nc.dram_tensor("out_loc", [NB, CS], F32, kind="ExternalOutput")

    with tile.TileContext(nc) as tc:
        with ExitStack() as ctx:
            _emit(ctx, tc, nc, sT_d, qsT_d, rot_full, trans_full, q_rot,
                  q_trans, wall_d, wout_d, qscale_d, out_loc, gw0, gw1)

    nc.compile()
    _PROGRAM_CACHE[key] = nc
    return nc


def _emit(ctx, tc, nc, sT_d, qsT_d, rot_full, trans_full, q_rot, q_trans,
          wall_d, wout_d, qscale_d, out_loc, gw0, gw1):
    PS = bass.MemorySpace.PSUM

    const = ctx.enter_context(tc.tile_pool(name="const", bufs=1))
    work = ctx.enter_context(tc.tile_pool(name="work", bufs=1))
    attA = ctx.enter_context(tc.tile_pool(name="attA", bufs=1))
    tmp_pool = ctx.enter_context(tc.tile_pool(name="tmp", bufs=2))
    pA_ctx = ExitStack()
    pA = pA_ctx.enter_context(tc.tile_pool(name="pA", bufs=1))
    pre_ctx = ExitStack()
    tpsum = pre_ctx.enter_context(tc.tile_pool(name="tpsum", bufs=2, space=PS))

    # ---- input DMA (consumption order) -----------------------------------
    qsT = [pA.tile([128, NB], BF16, name=f"qsT{kc}") for kc in range(3)]
    for kc in range(3):
        nc.sync.dma_start(qsT[kc][:], qsT_d[kc * 128:(kc + 1) * 128, :])
    wall_sb = []
    for kc in range(3):
        t = pA.tile([128, WALL_COLS], BF16, name=f"wall{kc}")
        nc.sync.dma_start(t[:], wall_d[kc * 128:(kc + 1) * 128, :])
        wall_sb.append(t)
    sT = [pA.tile([128, N], BF16, name=f"sT{kc}") for kc in range(3)]
    for kc in range(3):
        nc.sync.dma_start(sT[kc][:], sT_d[kc * 128:(kc + 1) * 128, :])
    rot_all = const.tile([128, KT * 9], F32, name="rot_all")
    nc.sync.dma_start(rot_all[:].rearrange("p (n c) -> p n c", c=9),
                      rot_full[:].rearrange("(n p) c -> p n c", p=128))
    rot_k = [rot_all[:, kt * 9:(kt + 1) * 9] for kt in range(KT)]
    trn_all = const.tile([128, KT * 3], F32, name="trn_all")
    nc.sync.dma_start(trn_all[:].rearrange("p (n c) -> p n c", c=3),
                      trans_full[:].rearrange("(n p) c -> p n c", p=128))
    qrot_all = const.tile([128, NT * 9], F32, name="qrot_all")
    nc.sync.dma_start(qrot_all[:].rearrange("p (n c) -> p n c", c=9),
                      q_rot[:].rearrange("(n p) c -> p n c", p=128))
    rot_sb = [qrot_all[:, nt * 9:(nt + 1) * 9] for nt in range(NT)]
    qtr_all = const.tile([128, NT * 3], F32, name="qtr_all")
    nc.sync.dma_start(qtr_all[:].rearrange("p (n c) -> p n c", c=3),
                      q_trans[:].rearrange("(n p) c -> p n c", p=128))
    trans_sb = [qtr_all[:, nt * 3:(nt + 1) * 3] for nt in range(NT)]
    qscale_sb = const.tile([128, FEAT * H], F32)
    nc.sync.dma_start(qscale_sb[:], qscale_d[:, :])

    # ---- constants -------------------------------------------------------
    ident = const.tile([128, 128], F32)
    make_identity(nc, ident[:])
    ident_r = const.tile([128, 128], F32R)
    nc.vector.tensor_copy(ident_r[:], ident[:])
    ones2_f32 = const.tile([2, NB], F32)
    nc.gpsimd.memset(ones2_f32[:], 1.0)

    # ---- K/V-side projections (8 key tiles, cols 0:1248) -----------------
    kvp_sb = [pA.tile([128, 864], BF16, name=f"kvp{kt}") for kt in range(KT)]
    kf = [pA.tile([128, FEAT * H], F32R, name=f"kf{kt}") for kt in range(KT)]
    kfG = [attA.tile([128, N], BF16, name=f"kfG{t}") for t in range(6)]
    va = [attA.tile([128, OCH * H], BF16, name=f"va{kt}") for kt in range(KT)]
    ppsum = pre_ctx.enter_context(tc.tile_pool(name="ppsum", bufs=2, space=PS))

    def proj_block(sTl, nsl, c0, c1, outs):
        ps = ppsum.tile([128, c1 - c0], F32, tag="proj", name="ps")
        for kc in range(3):
            nc.tensor.matmul(ps[:], sTl[kc][:, nsl], wall_sb[kc][:, c0:c1],
                             start=(kc == 0), stop=(kc == 2))
        for (lo, hi, dv, kind) in outs:
            src = ps[:, lo - c0:hi - c0]
            if kind == "copy":
                nc.vector.tensor_copy(dv, src)
            elif kind == "sigmoid":
                nc.scalar.activation(dv, src, AF.Sigmoid)
            else:
                nc.scalar.activation(dv, src, AF.Relu)

    # ---- paired rigid transform ------------------------------------------
    # src layout per point: [c0 d0 c1 d1 c2 d2]; pair j = cols 2j:2j+2.
    # out pair i gets sum_j pv[:, :, :, j, :] * rt[j, i]; coords then -= 0
    # / += trans via a second pass on the coord half only.
    def emit_tf(cosrc, disrc, codst, didst, rt, tr, s1=False):
        """cosrc(j)/disrc(j): [128, H, x] views; codst/didst[i] same shape.
        s1=True puts the first madd of each output on the scalar engine."""
        for i in range(3):
            dv = codst[i]
            nc.vector.tensor_scalar(dv, cosrc(0), rt[:, 3 * i:3 * i + 1],
                                    tr[:, i:i + 1], OP.mult, OP.add)
            nc.vector.scalar_tensor_tensor(dv, cosrc(1),
                                           rt[:, 3 * i + 1:3 * i + 2], dv,
                                           OP.mult, OP.add)
            nc.vector.scalar_tensor_tensor(dv, cosrc(2),
                                           rt[:, 3 * i + 2:3 * i + 3], dv,
                                           OP.mult, OP.add)
            dv = didst[i]
            if s1:
                nc.scalar.activation(dv, disrc(0), AF.Copy,
                                     scale=rt[:, 3 * i:3 * i + 1])
            else:
                nc.vector.tensor_scalar_mul(dv, disrc(0),
                                            rt[:, 3 * i:3 * i + 1])
            nc.vector.scalar_tensor_tensor(dv, disrc(1),
                                           rt[:, 3 * i + 1:3 * i + 2], dv,
                                           OP.mult, OP.add)
            nc.vector.scalar_tensor_tensor(dv, disrc(2),
                                           rt[:, 3 * i + 2:3 * i + 3], dv,
                                           OP.mult, OP.add)

    def sumsq(eng, dst, t2, cviews):
        eng.tensor_tensor(dst[:], cviews[0], cviews[0], OP.mult)
        for cv in cviews[1:]:
            eng.tensor_tensor(t2[:], cv, cv, OP.mult)
            eng.tensor_tensor(dst[:], dst[:], t2[:], OP.add)

    def psum4(eng, dst, srct):
        sv = srct[:].rearrange("p (h x) -> p h x", x=P)
        eng.tensor_tensor(dst, sv[:, :, 0], sv[:, :, 1], OP.add)
        eng.tensor_tensor(dst, dst, sv[:, :, 2], OP.add)
        eng.tensor_tensor(dst, dst, sv[:, :, 3], OP.add)

    expT_tiles = [work.tile([128, 2048], BF16, name=f"expT{i}")
                  for i in range(H)]
    qpsum = pre_ctx.enter_context(tc.tile_pool(name="qpsum", bufs=2, space=PS))

    # ---- Q-side ----------------------------------------------------------
    g_sb = [work.tile([128, 48], F32, name=f"g{nt}") for nt in range(NT)]
    qp_sb = [work.tile([128, 288], BF16, name=f"qp{nt}") for nt in range(NT)]
    qkvp_sb = [work.tile([128, 864], BF16, name=f"qkvp{nt}") for nt in range(NT)]
    qf = [work.tile([128, FEAT * H], F32, name=f"qf{nt}") for nt in range(NT)]
    qsv = qscale_sb[:].rearrange("p (h f) -> p h f", f=FEAT)
    for nt in range(NT):
        nsl = slice(nt * 128, (nt + 1) * 128)
        qfv = qf[nt][:].rearrange("p (h f) -> p h f", f=FEAT)
        proj_block(qsT, nsl, Q_OFF, Q_OFF + 240,
                   [(Q_OFF, Q_OFF + 192, qfv[:, :, 0:16], "copy"),
                    (G_OFF, G_OFF + 48, g_sb[nt][:], "sigmoid")])
        proj_block(qsT, nsl, QP_OFF, QP_OFF + 288,
                   [(QP_OFF, QP_OFF + 288, qp_sb[nt][:], "relu")])
        proj_block(qsT, nsl, KVP_OFF, KVP_OFF + 432,
                   [(KVP_OFF, KVP_OFF + 432, qkvp_sb[nt][:, 0:432], "relu")])
        proj_block(qsT, nsl, KVP_OFF + 432, KVP_OFF + 864,
                   [(KVP_OFF + 432, KVP_OFF + 864, qkvp_sb[nt][:, 432:864],
                     "relu")])

    for nt in range(NT):
        qfv = qf[nt][:].rearrange("p (h f) -> p h f", f=FEAT)
        qpv = qp_sb[nt][:].rearrange("p (h x c) -> p h x c", x=4, c=6)
        qf_coords = [qfv[:, :, 16 + 8 * i:20 + 8 * i] for i in range(3)]
        qf_dirs = [qfv[:, :, 20 + 8 * i:24 + 8 * i] for i in range(3)]
        emit_tf(lambda j: qpv[:, :, :, j], lambda j: qpv[:, :, :, 3 + j],
                qf_coords, qf_dirs, rot_sb[nt], trans_sb[nt])
        # gate all 6 comps (coords+dirs)
        gv = g_sb[nt][:].rearrange("p (h x) -> p h x", x=P)
        for i in range(6):
            half = qfv[:, :, 16 + 4 * i:20 + 4 * i]
            nc.vector.tensor_tensor(half, half, gv, OP.mult)

    # k-dirs for q rows (curvature), dirs only
    pco_qk = [work.tile([128, 3 * 48], F32, name=f"pqk{nt}") for nt in range(NT)]
    for nt in range(NT):
        pv = qkvp_sb[nt][:].rearrange("p (h x c) -> p h x c", x=12, c=6)
        rt = rot_sb[nt]
        for i in range(3):
            ddi = pco_qk[nt][:, i * 48:(i + 1) * 48].rearrange(
                "p (h x) -> p h x", x=4)
            nc.vector.tensor_scalar_mul(ddi, pv[:, :, 0:4, 3],
                                        rt[:, 3 * i:3 * i + 1])
            nc.vector.scalar_tensor_tensor(ddi, pv[:, :, 0:4, 4],
                                           rt[:, 3 * i + 1:3 * i + 2], ddi,
                                           OP.mult, OP.add)
            nc.vector.scalar_tensor_tensor(ddi, pv[:, :, 0:4, 5],
                                           rt[:, 3 * i + 2:3 * i + 3], ddi,
                                           OP.mult, OP.add)

    def comp_qk(nt, i):
        return pco_qk[nt][:, i * 48:(i + 1) * 48].rearrange(
            "p (h x) -> p h x", x=4)

    # ---- q-side reductions into qf cols 40/41 ----------------------------
    for nt in range(NT):
        qfv = qf[nt][:].rearrange("p (h f) -> p h f", f=FEAT)
        qf_coords = [qfv[:, :, 16 + 8 * i:20 + 8 * i] for i in range(3)]
        qf_dirs = [qfv[:, :, 20 + 8 * i:24 + 8 * i] for i in range(3)]
        eng = nc.gpsimd if nt == 0 else nc.vector
        sq = tmp_pool.tile([128, 48], F32, tag="sq", name="sq")
        t2 = tmp_pool.tile([128, 48], F32, tag="t2", name="t2")
        sumsq(eng, sq, t2, qf_coords)
        psum4(eng, qfv[:, :, 41], sq)

        cr = tmp_pool.tile([128, 48], F32, tag="cr", name="cr")
        cs_ = tmp_pool.tile([128, 48], F32, tag="cs_", name="cs_")
        t3 = tmp_pool.tile([128, 48], F32, tag="t3", name="t3")
        first = True
        for (a, b_) in ((1, 2), (2, 0), (0, 1)):
            eng.tensor_tensor(cr[:], qf_dirs[a], comp_qk(nt, b_), OP.mult)
            eng.tensor_tensor(t3[:], qf_dirs[b_], comp_qk(nt, a), OP.mult)
            eng.tensor_tensor(cr[:], cr[:], t3[:], OP.subtract)
            eng.tensor_tensor(cr[:], cr[:], cr[:], OP.mult)
            if first:
                eng.tensor_copy(cs_[:], cr[:])
                first = False
            else:
                eng.tensor_tensor(cs_[:], cs_[:], cr[:], OP.add)
        nq2 = tmp_pool.tile([128, 48], F32, tag="nq2", name="nq2")
        nk2 = tmp_pool.tile([128, 48], F32, tag="nk2", name="nk2")
        sumsq(eng, nq2, t2, qf_dirs)
        sumsq(eng, nk2, t2, [comp_qk(nt, i) for i in range(3)])
        eng.tensor_tensor(nq2[:], nq2[:], nk2[:], OP.mult)
        nc.scalar.activation(nq2[:], nq2[:], AF.Sqrt)
        nc.vector.tensor_scalar_add(nq2[:], nq2[:], EPS)
        nc.vector.reciprocal(nq2[:], nq2[:])
        nc.scalar.activation(cs_[:], cs_[:], AF.Sqrt,
                             scale=float((gw1 / gw0) ** 2))
        eng.tensor_tensor(cs_[:], cs_[:], nq2[:], OP.mult)
        csv = cs_[:].rearrange("p (h x) -> p h x", x=P)
        for x in range(P):
            eng.tensor_tensor(qfv[:, :, 41], qfv[:, :, 41],
                              csv[:, :, x], OP.add)
        eng.tensor_tensor(qfv[:, :, 41], qfv[:, :, 41],
                          qsv[:, :, 40], OP.mult)
        eng.tensor_copy(qfv[:, :, 40], qsv[:, :, 40])

    # warm the EXP table off the critical path
    warm = tmp_pool.tile([2, 2], F32, tag="warm", name="warm")
    nc.vector.memset(warm[:], 0.0)
    nc.scalar.activation(warm[:], warm[:], AF.Exp)

    for nt in range(NT):
        qfv = qf[nt][:].rearrange("p (h f) -> p h f", f=FEAT)
        nc.vector.tensor_tensor(qfv[:, :, 0:40], qfv[:, :, 0:40],
                                qsv[:, :, 0:40], OP.mult)

    qfT = [work.tile([128, NB], BF16, name=f"qfT{t}") for t in range(6)]
    for t in range(6):
        for nt in range(NT):
            ps = tpsum.tile([128, 128], F32, tag="tps")
            nc.tensor.transpose(ps[:], qf[nt][:, t * 128:(t + 1) * 128], ident[:])
            if t % 2:
                nc.scalar.copy(qfT[t][:, nt * 128:(nt + 1) * 128], ps[:])
            else:
                nc.vector.tensor_copy(qfT[t][:, nt * 128:(nt + 1) * 128], ps[:])

    for kt in range(KT):
        ksl = slice(kt * 128, (kt + 1) * 128)
        kfv = kf[kt][:].rearrange("p (h f) -> p h f", f=FEAT)
        vav = va[kt][:].rearrange("p (h f) -> p h f", f=OCH)
        proj_block(sT, ksl, 0, 512,
                   [(0, 192, kfv[:, :, 0:16], "copy"),
                    (192, 384, vav[:, :, 0:16], "copy"),
                    (384, 512, kvp_sb[kt][:, 0:128], "relu")])
        proj_block(sT, ksl, 512, 1024,
                   [(512, 1024, kvp_sb[kt][:, 128:640], "relu")])
        proj_block(sT, ksl, 1024, 1248,
                   [(1024, 1248, kvp_sb[kt][:, 640:864], "relu")])

    for kt in range(KT):
        kfv = kf[kt][:].rearrange("p (h f) -> p h f", f=FEAT)
        vav = va[kt][:].rearrange("p (h f) -> p h f", f=OCH)
        pv_h = kvp_sb[kt][:].rearrange("p (h x c) -> p h x c", x=12, c=6)
        kf_coords = [kfv[:, :, 16 + 8 * i:20 + 8 * i] for i in range(3)]
        kf_dirs = [kfv[:, :, 20 + 8 * i:24 + 8 * i] for i in range(3)]
        emit_tf(lambda j: pv_h[:, :, 0:4, j], lambda j: pv_h[:, :, 0:4, 3 + j],
                kf_coords, kf_dirs, rot_k[kt],
                trn_all[:, kt * 3:(kt + 1) * 3])
        va_coords = [vav[:, :, 16 + 16 * i:24 + 16 * i] for i in range(3)]
        va_dirs = [vav[:, :, 24 + 16 * i:32 + 16 * i] for i in range(3)]
        emit_tf(lambda j: pv_h[:, :, 4:12, j], lambda j: pv_h[:, :, 4:12, 3 + j],
                va_coords, va_dirs, rot_k[kt],
                trn_all[:, kt * 3:(kt + 1) * 3], s1=True)
        # k2: sum of squared coords over pts and comps
        sq = tmp_pool.tile([128, 48], F32, tag="sq", name="sq")
        t2 = tmp_pool.tile([128, 48], F32, tag="t2", name="t2")
        sumsq(nc.gpsimd, sq, t2, kf_coords)
        psum4(nc.gpsimd, kfv[:, :, 40], sq)
        nc.gpsimd.memset(kfv[:, :, 41].bitcast(F32), 1.0)
        nc.gpsimd.memset(vav[:, :, 64], 1.0)
        nc.gpsimd.memset(vav[:, :, 65], 0.0)
        for t in range(6):
            ps = tpsum.tile([128, 128], F32R, tag="tpsr")
            nc.tensor.transpose(ps[:], kf[kt][:, t * 128:(t + 1) * 128],
                                ident_r[:])
            if (kt + t) % 2:
                nc.scalar.copy(kfG[t][:, kt * 128:(kt + 1) * 128],
                               ps[:].bitcast(F32))
            else:
                nc.vector.tensor_copy(kfG[t][:, kt * 128:(kt + 1) * 128],
                                      ps[:].bitcast(F32))
        # fused QK + EXP for this key tile, all heads
        for h in range(H):
            t, base = h // 2, (h % 2) * 64
            aps = qpsum.tile([128, NB], F32, tag="qk", name="aps")
            nc.tensor.matmul(aps[:],
                             kfG[t][base:base + FS, kt * 128:(kt + 1) * 128],
                             qfT[t][base:base + FS, :],
                             start=True, stop=True)
            nc.scalar.activation(
                expT_tiles[h][:, kt * NB:(kt + 1) * NB], aps[:], AF.Exp)

    # ---- inverse transform helper ----------------------------------------
    feats = [work.tile([128, FOUT], F32, name=f"feats{qt}") for qt in range(NT)]
    tinv = [work.tile([128, 3], F32, name=f"tinv{qt}") for qt in range(NT)]
    for qt in range(NT):
        rt, tr = rot_sb[qt], trans_sb[qt]
        for i in range(3):
            nc.vector.tensor_scalar_mul(tinv[qt][:, i:i + 1], tr[:, 0:1],
                                        rt[:, i:i + 1])
            nc.vector.scalar_tensor_tensor(tinv[qt][:, i:i + 1], tr[:, 1:2],
                                           rt[:, 3 + i:4 + i], tinv[qt][:, i:i + 1],
                                           OP.mult, OP.add)
            nc.vector.scalar_tensor_tensor(tinv[qt][:, i:i + 1], tr[:, 2:3],
                                           rt[:, 6 + i:7 + i], tinv[qt][:, i:i + 1],
                                           OP.mult, OP.add)

    def emit_inverse(qt, hh):
        hs = slice(hh * 6, hh * 6 + 6)
        ovv = o_all[qt][:].rearrange("p (h f) -> p h f", f=FEAT)[:, hs]

        def ogp(j):
            return ovv[:, :, 16 + 16 * j:32 + 16 * j]

        nc.vector.tensor_copy(
            feats[qt][:, hh * 96:hh * 96 + 96].rearrange("p (h c) -> p h c", c=16),
            ovv[:, :, 0:16])
        gview = feats[qt][:, 192:FOUT].rearrange(
            "p (h x c) -> p h x c", h=H, c=7)[:, hs]
        rt = rot_sb[qt]

        lcld = [tmp_pool.tile([128, 96], F32, tag=f"lcld{i}", name=f"lcld{i}")
                for i in range(3)]
        for i in range(3):
            lv = lcld[i][:].rearrange("p (h x) -> p h x", x=16)
            nc.vector.tensor_scalar_mul(lv, ogp(0), rt[:, i:i + 1])
            nc.vector.scalar_tensor_tensor(lv, ogp(1), rt[:, 3 + i:4 + i],
                                           lv, OP.mult, OP.add)
            nc.vector.scalar_tensor_tensor(lv, ogp(2), rt[:, 6 + i:7 + i],
                                           lv, OP.mult, OP.add)
            nc.vector.tensor_scalar(lv[:, :, 0:8], lv[:, :, 0:8],
                                    tinv[qt][:, i:i + 1], None, OP.subtract)
        n2 = tmp_pool.tile([128, 96], F32, tag="n2", name="n2")
        t2b = tmp_pool.tile([128, 96], F32, tag="t2b", name="t2b")
        nc.gpsimd.tensor_tensor(n2[:], lcld[0][:], lcld[0][:], OP.mult)
        for i in (1, 2):
            nc.gpsimd.tensor_tensor(t2b[:], lcld[i][:], lcld[i][:], OP.mult)
            nc.gpsimd.tensor_tensor(n2[:], n2[:], t2b[:], OP.add)
        n2v = n2[:].rearrange("p (h x) -> p h x", x=16)
        nc.scalar.activation(gview[:, :, :, 6], n2v[:, :, 0:8], AF.Sqrt)
        nc.scalar.activation(n2v[:, :, 8:16], n2v[:, :, 8:16], AF.Sqrt)
        ndv = n2v[:, :, 8:16]
        nc.vector.tensor_scalar_max(ndv, ndv, EPS)
        nc.vector.reciprocal(ndv, ndv)
        for i in range(3):
            lv = lcld[i][:].rearrange("p (h x) -> p h x", x=16)
            nc.gpsimd.tensor_copy(gview[:, :, :, i], lv[:, :, 0:8])
            nc.gpsimd.tensor_tensor(gview[:, :, :, 3 + i],
                                    lv[:, :, 8:16], ndv, OP.mult)

    # ---- attention AV (QK/EXP ran fused in the K-side loop) --------------
    pre_ctx.close()
    pA_ctx.close()
    att_ctx = ExitStack()
    opsum = att_ctx.enter_context(tc.tile_pool(name="opsum", bufs=2, space=PS))
    o_all = [work.tile([128, FEAT * H], F32, name=f"oall{qt}") for qt in range(NT)]

    def emit_av(h):
        expT = expT_tiles[h]
        ot_ps = opsum.tile([OCH, NB], F32, tag="otacc", name="ot_ps")
        for kb in range(NKB):
            nc.tensor.matmul(
                ot_ps[:],
                va[kb][:, h * OCH:(h + 1) * OCH],
                expT[:, kb * NB:(kb + 1) * NB],
                start=(kb == 0), stop=(kb == NKB - 1))
        ot_sb = tmp_pool.tile([OCH, NB], F32R, tag="otsb", name="otsb", bufs=2)
        nc.vector.tensor_copy(ot_sb[:], ot_ps[:])
        for qt in range(NT):
            tp = opsum.tile([128, OCH], F32R, tag="otp", name="tp")
            nc.tensor.transpose(tp[:], ot_sb[:, qt * 128:(qt + 1) * 128],
                                ident_r[0:OCH, 0:OCH])
            rec = tmp_pool.tile([128, 1], F32, tag="rec", name="rec")
            nc.vector.reciprocal(rec[:], tp[:, 64:65].bitcast(F32))
            nc.vector.tensor_scalar_mul(
                o_all[qt][:, h * FEAT:h * FEAT + 64], tp[:, 0:64].bitcast(F32),
                rec[:])

    for h in range(H):
        emit_av(h)
        if h == 6:
            for qt in range(NT):
                emit_inverse(qt, 0)
    for qt in range(NT):
        emit_inverse(qt, 1)

    # ---- output projection -----------------------------------------------
    att_ctx.close()
    wout_sb = []
    for kc in range(KCH):
        r0 = kc * 128
        r1 = min(FOUT + 2, r0 + 128)
        t = const.tile([r1 - r0, CS], BF16, name=f"wout{kc}")
        nc.sync.dma_start(t[:], wout_d[r0:r1, :])
        wout_sb.append(t)
    tpsum2 = ctx.enter_context(tc.tile_pool(name="tpsum2", bufs=2, space=PS))
    opsum2 = ctx.enter_context(tc.tile_pool(name="opsum2", bufs=2, space=PS))
    fT = []
    for kc in range(KCH):
        r0 = kc * 128
        rw = min(FOUT, r0 + 128) - r0
        pw = rw + 2 if kc == KCH - 1 else rw
        t = work.tile([pw, NB], BF16, name=f"fT{kc}")
        fT.append(t)
    lastr = FOUT - (KCH - 1) * 128
    nc.vector.tensor_copy(fT[KCH - 1][lastr:lastr + 2, :], ones2_f32[:])
    for kc in range(KCH):
        r0 = kc * 128
        rw = min(FOUT, r0 + 128) - r0
        for qt in range(NT):
            ps = tpsum2.tile([128, 128], F32, tag="tps2")
            nc.tensor.transpose(ps[:rw, :], feats[qt][:, r0:r0 + rw], ident[:])
            if kc % 2:
                nc.scalar.copy(fT[kc][:rw, qt * 128:(qt + 1) * 128], ps[:rw, :])
            else:
                nc.vector.tensor_copy(fT[kc][:rw, qt * 128:(qt + 1) * 128],
                                      ps[:rw, :])

    for qt in range(NT):
        ps = opsum2.tile([128, CS], F32, tag="oproj")
        for kc in range(KCH):
            nc.tensor.matmul(ps[:], fT[kc][:, qt * 128:(qt + 1) * 128],
                             wout_sb[kc][:], start=(kc == 0), stop=(kc == KCH - 1))
        osb = tmp_pool.tile([128, CS], F32, tag="osb", name="osb")
        nc.scalar.copy(osb[:], ps[:])
        nc.sync.dma_start(out_loc[qt * 128:(qt + 1) * 128, :], osb[:])


def _run(inputs, trace=False):
    (s, sT, rot9, trans, wall, wout_b, qscale, bfbits, gw) = _host_prep(inputs)
    nc = _build_program(float(gw[0]), float(gw[1]))
    in_maps = []
    for c in range(8):
        b, qb = c // 4, c % 4
        r = slice(qb * NB, (qb + 1) * NB)
        in_maps.append({
            "sT_d": bfbits(sT[b]),
            "qsT_d": bfbits(sT[b][:, r]),
            "rot_full": np.ascontiguousarray(rot9[b]),
            "trans_full": np.ascontiguousarray(trans[b]),
            "q_rot": np.ascontiguousarray(rot9[b, r]),
            "q_trans": np.ascontiguousarray(trans[b, r]),
            "wall": wall, "wout_b": wout_b, "qscale": qscale,
        })
    res = run_bass_kernel_spmd(nc, in_maps, list(range(8)), trace=trace)
    out = np.empty((B, N, CS), np.float32)
    for c in range(8):
        b, qb = c // 4, c % 4
        out[b, qb * NB:(qb + 1) * NB] = res.results[c]["out_loc"]
    return out, res


def kernel(**inputs):
    out, _ = _run(inputs, trace=False)
    return out


def kernel_traced(**inputs):
    return _run(inputs, trace=True)


# revision 50
# speedup vs baseline: 1.2129x; 1.1844x over previous
"""Bass/Tile TRN2 kernel for EnhancedIPA3 (invariant-point-attention variant).

Sharding: 8 cores = batch(2) x query-block(4).  NO collectives: each core
recomputes the full K/V side for its batch locally (deterministic; the
AllGather variant was hostage to inter-core launch skew).  Query side runs
on the core's own 256 rows (separately-sliced inputs).

Point channels are pair-interleaved [c0 d0 c1 d1 c2 d2] per point so each
rigid-transform madd covers coord+dir at double width, and the transforms
write straight into the attention feature tiles (kf/va/qf) -- no staging
buffers or assembly copies.  s arrives host-pretransposed in bf16.  All
biases are zero per the problem spec and are dropped.

Self-contained: hardcodes all shapes; only depends on numpy + concourse.
"""

import numpy as np
from contextlib import ExitStack

import concourse.bass as bass
import concourse.bacc as bacc
import concourse.mybir as mybir
import concourse.tile as tile
from concourse.bass_utils import run_bass_kernel_spmd
from concourse.masks import make_identity

F32 = mybir.dt.float32
F32R = mybir.dt.float32r
BF16 = mybir.dt.bfloat16
AF = mybir.ActivationFunctionType
OP = mybir.AluOpType

B, N, CS, H, C, P, V = 2, 1024, 384, 12, 16, 4, 8
EPS = 1e-8
NB = N // 4            # 256 query rows per core
NT = NB // 128
KT = N // 128          # 8 key row-tiles (full batch, local)
K_OFF, V_OFF, KVP_OFF = 0, 192, 384
Q_OFF, G_OFF, QP_OFF = 1248, 1440, 1488
WALL_COLS = 1776
FEAT = 64              # per-head feature stride in kf/qf
FS = 42
OCH = 66
FOUT = H * (C + 7 * V)
KCH = 7
NKB = 8


def _host_prep(inputs):
    wq = inputs["wq"]
    wkv = inputs["wkv"]
    wqp = inputs["wqp"]
    wkvp = inputs["wkvp"]
    wg = inputs["wg"]
    gw = np.asarray(inputs["geom_weight"], np.float32)
    hw = np.asarray(inputs["head_weights"], np.float32)
    sh = 1.0 / (1.0 + np.exp(-hw))

    wall = np.zeros((CS, WALL_COLS), np.float32)
    wall[:, K_OFF:K_OFF + 192] = wkv[:, :192]
    wall[:, V_OFF:V_OFF + 192] = wkv[:, 192:]
    wall[:, KVP_OFF:KVP_OFF + 864] = wkvp
    wall[:, Q_OFF:Q_OFF + 192] = wq
    wall[:, G_OFF:G_OFF + 48] = wg
    wall[:, QP_OFF:QP_OFF + 288] = wqp

    bout_half = np.asarray(inputs["bout"], np.float32)[None, :] * 0.5
    wout_b = np.concatenate(
        [np.asarray(inputs["wout"], np.float32), bout_half, bout_half],
        axis=0)

    # qf/kf feature layout per head (FEAT=64 stride):
    #  [0:16] scalar q/k; [16+8i : 20+8i] coords_i (4 pts); [20+8i : 24+8i]
    #  dirs_i; [40] k2 / const; [41] q-combo; rest pad
    qs = np.zeros((FEAT * H,), np.float32)
    for h in range(H):
        o = h * FEAT
        qs[o:o + 16] = sh[h] / np.sqrt(C)
        for i in range(3):
            qs[o + 16 + 8 * i:o + 20 + 8 * i] = sh[h] * gw[0] * 0.5
            qs[o + 20 + 8 * i:o + 24 + 8 * i] = sh[h] * gw[1]
        qs[o + 40] = -sh[h] * gw[0] / P
        qs[o + 41] = 1.0
    qscale = np.broadcast_to(qs, (128, FEAT * H)).copy()

    rot9 = np.ascontiguousarray(
        np.asarray(inputs["rot"], np.float32).reshape(B, N, 9))
    trans = np.asarray(inputs["trans"], np.float32)
    s = np.asarray(inputs["s"], np.float32)

    def bfbits(a):
        u = np.ascontiguousarray(a, np.float32).view(np.uint32)
        r = ((u >> 16) + ((u >> 15) & 1)).astype(np.uint32)
        return (r & 0xFFFF).astype(np.uint16)

    sT = np.ascontiguousarray(s.transpose(0, 2, 1))      # [B, CS, N]
    return (s, sT, rot9, trans, bfbits(wall), bfbits(wout_b), qscale,
            bfbits, gw)


_PROGRAM_CACHE = {}


def _build_program(gw0, gw1):
    key = (float(gw0), float(gw1))
    if key in _PROGRAM_CACHE:
        return _PROGRAM_CACHE[key]

    nc = bacc.Bacc("TRN2", target_bir_lowering=False, debug=False, num_devices=8)

    sT_d = nc.dram_tensor("sT_d", [CS, N], BF16, kind="ExternalInput")
    qsT_d = nc.dram_tensor("qsT_d", [CS, NB], BF16, kind="ExternalInput")
    rot_full = nc.dram_tensor("rot_full", [N, 9], F32, kind="ExternalInput")
    trans_full = nc.dram_tensor("trans_full", [N, 3], F32, kind="ExternalInput")
    q_rot = nc.dram_tensor("q_rot", [NB, 9], F32, kind="ExternalInput")
    q_trans = nc.dram_tensor("q_trans", [NB, 3], F32, kind="ExternalInput")
    wall_d = nc.dram_tensor("wall", [CS, WALL_COLS], BF16, kind="ExternalInput")
    wout_d = nc.dram_tensor("wout_b", [FOUT + 2, CS], BF16, kind="ExternalInput")
    qscale_d = nc.dram_tensor("qscale", [128, FEAT * H], F32, kind="ExternalInput")
    out_loc = nc.dram_tensor("out_loc", [NB, CS], F32, kind="ExternalOutput")

    with tile.TileContext(nc) as tc:
        with ExitStack() as ctx:
            _emit(ctx, tc, nc, sT_d, qsT_d, rot_full, trans_full, q_rot,
                  q_trans, wall_d, wout_d, qscale_d, out_loc, gw0, gw1)

    nc.compile()
    _PROGRAM_CACHE[key] = nc
    return nc


def _emit(ctx, tc, nc, sT_d, qsT_d, rot_full, trans_full, q_rot, q_trans,
          wall_d, wout_d, qscale_d, out_loc, gw0, gw1):
    PS = bass.MemorySpace.PSUM

    const = ctx.enter_context(tc.tile_pool(name="const", bufs=1))
    work = ctx.enter_context(tc.tile_pool(name="work", bufs=1))
    attA = ctx.enter_context(tc.tile_pool(name="attA", bufs=1))
    tmp_pool = ctx.enter_context(tc.tile_pool(name="tmp", bufs=2))
    pA_ctx = ExitStack()
    pA = pA_ctx.enter_context(tc.tile_pool(name="pA", bufs=1))
    pre_ctx = ExitStack()
    tpsum = pre_ctx.enter_context(tc.tile_pool(name="tpsum", bufs=2, space=PS))

    # ---- input DMA (consumption order) -----------------------------------
    qsT = [pA.tile([128, NB], BF16, name=f"qsT{kc}") for kc in range(3)]
    for kc in range(3):
        nc.sync.dma_start(qsT[kc][:], qsT_d[kc * 128:(kc + 1) * 128, :])
    wall_sb = []
    for kc in range(3):
        t = pA.tile([128, WALL_COLS], BF16, name=f"wall{kc}")
        nc.sync.dma_start(t[:], wall_d[kc * 128:(kc + 1) * 128, :])
        wall_sb.append(t)
    sT = [pA.tile([128, N], BF16, name=f"sT{kc}") for kc in range(3)]
    for kc in range(3):
        nc.sync.dma_start(sT[kc][:], sT_d[kc * 128:(kc + 1) * 128, :])
    rot_all = const.tile([128, KT * 9], F32, name="rot_all")
    nc.sync.dma_start(rot_all[:].rearrange("p (n c) -> p n c", c=9),
                      rot_full[:].rearrange("(n p) c -> p n c", p=128))
    rot_k = [rot_all[:, kt * 9:(kt + 1) * 9] for kt in range(KT)]
    trn_all = const.tile([128, KT * 3], F32, name="trn_all")
    nc.sync.dma_start(trn_all[:].rearrange("p (n c) -> p n c", c=3),
                      trans_full[:].rearrange("(n p) c -> p n c", p=128))
    qrot_all = const.tile([128, NT * 9], F32, name="qrot_all")
    nc.sync.dma_start(qrot_all[:].rearrange("p (n c) -> p n c", c=9),
                      q_rot[:].rearrange("(n p) c -> p n c", p=128))
    rot_sb = [qrot_all[:, nt * 9:(nt + 1) * 9] for nt in range(NT)]
    qtr_all = const.tile([128, NT * 3], F32, name="qtr_all")
    nc.sync.dma_start(qtr_all[:].rearrange("p (n c) -> p n c", c=3),
                      q_trans[:].rearrange("(n p) c -> p n c", p=128))
    trans_sb = [qtr_all[:, nt * 3:(nt + 1) * 3] for nt in range(NT)]
    qscale_sb = const.tile([128, FEAT * H], F32)
    nc.sync.dma_start(qscale_sb[:], qscale_d[:, :])

    # ---- constants -------------------------------------------------------
    ident = const.tile([128, 128], F32)
    make_identity(nc, ident[:])
    ident_r = const.tile([128, 128], F32R)
    nc.vector.tensor_copy(ident_r[:], ident[:])
    ones2_f32 = const.tile([2, NB], F32)
    nc.gpsimd.memset(ones2_f32[:], 1.0)

    # ---- K/V-side projections (8 key tiles, cols 0:1248) -----------------
    kvp_sb = [pA.tile([128, 864], BF16, name=f"kvp{kt}") for kt in range(KT)]
    kf = [pA.tile([128, FEAT * H], F32R, name=f"kf{kt}") for kt in range(KT)]
    kfG = [attA.tile([128, N], BF16, name=f"kfG{t}") for t in range(6)]
    va = [attA.tile([128, OCH * H], BF16, name=f"va{kt}") for kt in range(KT)]
    ppsum = pre_ctx.enter_context(tc.tile_pool(name="ppsum", bufs=2, space=PS))

    def proj_block(sTl, nsl, c0, c1, outs):
        ps = ppsum.tile([128, c1 - c0], F32, tag="proj", name="ps")
        for kc in range(3):
            nc.tensor.matmul(ps[:], sTl[kc][:, nsl], wall_sb[kc][:, c0:c1],
                             start=(kc == 0), stop=(kc == 2))
        for (lo, hi, dv, kind) in outs:
            src = ps[:, lo - c0:hi - c0]
            if kind == "copy":
                nc.vector.tensor_copy(dv, src)
            elif kind == "sigmoid":
                nc.scalar.activation(dv, src, AF.Sigmoid)
            else:
                nc.scalar.activation(dv, src, AF.Relu)

    # ---- paired rigid transform ------------------------------------------
    # src layout per point: [c0 d0 c1 d1 c2 d2]; pair j = cols 2j:2j+2.
    # out pair i gets sum_j pv[:, :, :, j, :] * rt[j, i]; coords then -= 0
    # / += trans via a second pass on the coord half only.
    def emit_tf(cosrc, disrc, codst, didst, rt, tr, s1=False):
        """cosrc(j)/disrc(j): [128, H, x] views; codst/didst[i] same shape.
        s1=True puts the first madd of each output on the scalar engine."""
        for i in range(3):
            dv = codst[i]
            nc.vector.tensor_scalar(dv, cosrc(0), rt[:, 3 * i:3 * i + 1],
                                    tr[:, i:i + 1], OP.mult, OP.add)
            nc.vector.scalar_tensor_tensor(dv, cosrc(1),
                                           rt[:, 3 * i + 1:3 * i + 2], dv,
                                           OP.mult, OP.add)
            nc.vector.scalar_tensor_tensor(dv, cosrc(2),
                                           rt[:, 3 * i + 2:3 * i + 3], dv,
                                           OP.mult, OP.add)
            dv = didst[i]
            if s1:
                nc.scalar.activation(dv, disrc(0), AF.Copy,
                                     scale=rt[:, 3 * i:3 * i + 1])
            else:
                nc.vector.tensor_scalar_mul(dv, disrc(0),
                                            rt[:, 3 * i:3 * i + 1])
            nc.vector.scalar_tensor_tensor(dv, disrc(1),
                                           rt[:, 3 * i + 1:3 * i + 2], dv,
                                           OP.mult, OP.add)
            nc.vector.scalar_tensor_tensor(dv, disrc(2),
                                           rt[:, 3 * i + 2:3 * i + 3], dv,
                                           OP.mult, OP.add)

    def sumsq(eng, dst, t2, cviews):
        eng.tensor_tensor(dst[:], cviews[0], cviews[0], OP.mult)
        for cv in cviews[1:]:
            eng.tensor_tensor(t2[:], cv, cv, OP.mult)
            eng.tensor_tensor(dst[:], dst[:], t2[:], OP.add)

    def psum4(eng, dst, srct):
        sv = srct[:].rearrange("p (h x) -> p h x", x=P)
        eng.tensor_tensor(dst, sv[:, :, 0], sv[:, :, 1], OP.add)
        eng.tensor_tensor(dst, dst, sv[:, :, 2], OP.add)
        eng.tensor_tensor(dst, dst, sv[:, :, 3], OP.add)

    expT_tiles = [work.tile([128, 2048], BF16, name=f"expT{i}")
                  for i in range(H)]
    qpsum = pre_ctx.enter_context(tc.tile_pool(name="qpsum", bufs=2, space=PS))

    # ---- Q-side ----------------------------------------------------------
    g_sb = [work.tile([128, 48], F32, name=f"g{nt}") for nt in range(NT)]
    qp_sb = [work.tile([128, 288], BF16, name=f"qp{nt}") for nt in range(NT)]
    qkvp_sb = [work.tile([128, 864], BF16, name=f"qkvp{nt}") for nt in range(NT)]
    qf = [work.tile([128, FEAT * H], F32, name=f"qf{nt}") for nt in range(NT)]
    qsv = qscale_sb[:].rearrange("p (h f) -> p h f", f=FEAT)
    for nt in range(NT):
        nsl = slice(nt * 128, (nt + 1) * 128)
        qfv = qf[nt][:].rearrange("p (h f) -> p h f", f=FEAT)
        proj_block(qsT, nsl, Q_OFF, Q_OFF + 240,
                   [(Q_OFF, Q_OFF + 192, qfv[:, :, 0:16], "copy"),
                    (G_OFF, G_OFF + 48, g_sb[nt][:], "sigmoid")])
        proj_block(qsT, nsl, QP_OFF, QP_OFF + 288,
                   [(QP_OFF, QP_OFF + 288, qp_sb[nt][:], "relu")])
        proj_block(qsT, nsl, KVP_OFF, KVP_OFF + 432,
                   [(KVP_OFF, KVP_OFF + 432, qkvp_sb[nt][:, 0:432], "relu")])
        proj_block(qsT, nsl, KVP_OFF + 432, KVP_OFF + 864,
                   [(KVP_OFF + 432, KVP_OFF + 864, qkvp_sb[nt][:, 432:864],
                     "relu")])

    for nt in range(NT):
        qfv = qf[nt][:].rearrange("p (h f) -> p h f", f=FEAT)
        qpv = qp_sb[nt][:].rearrange("p (h x c) -> p h x c", x=4, c=6)
        qf_coords = [qfv[:, :, 16 + 8 * i:20 + 8 * i] for i in range(3)]
        qf_dirs = [qfv[:, :, 20 + 8 * i:24 + 8 * i] for i in range(3)]
        emit_tf(lambda j: qpv[:, :, :, j], lambda j: qpv[:, :, :, 3 + j],
                qf_coords, qf_dirs, rot_sb[nt], trans_sb[nt])
        # gate all 6 comps (coords+dirs)
        gv = g_sb[nt][:].rearrange("p (h x) -> p h x", x=P)
        for i in range(6):
            half = qfv[:, :, 16 + 4 * i:20 + 4 * i]
            nc.vector.tensor_tensor(half, half, gv, OP.mult)

    # k-dirs for q rows (curvature), dirs only
    pco_qk = [work.tile([128, 3 * 48], F32, name=f"pqk{nt}") for nt in range(NT)]
    for nt in range(NT):
        pv = qkvp_sb[nt][:].rearrange("p (h x c) -> p h x c", x=12, c=6)
        rt = rot_sb[nt]
        for i in range(3):
            ddi = pco_qk[nt][:, i * 48:(i + 1) * 48].rearrange(
                "p (h x) -> p h x", x=4)
            nc.vector.tensor_scalar_mul(ddi, pv[:, :, 0:4, 3],
                                        rt[:, 3 * i:3 * i + 1])
            nc.vector.scalar_tensor_tensor(ddi, pv[:, :, 0:4, 4],
                                           rt[:, 3 * i + 1:3 * i + 2], ddi,
                                           OP.mult, OP.add)
            nc.vector.scalar_tensor_tensor(ddi, pv[:, :, 0:4, 5],
                                           rt[:, 3 * i + 2:3 * i + 3], ddi,
                                           OP.mult, OP.add)

    def comp_qk(nt, i):
        return pco_qk[nt][:, i * 48:(i + 1) * 48].rearrange(
            "p (h x) -> p h x", x=4)

    # ---- q-side reductions into qf cols 40/41 ----------------------------
    for nt in range(NT):
        qfv = qf[nt][:].rearrange("p (h f) -> p h f", f=FEAT)
        qf_coords = [qfv[:, :, 16 + 8 * i:20 + 8 * i] for i in range(3)]
        qf_dirs = [qfv[:, :, 20 + 8 * i:24 + 8 * i] for i in range(3)]
        eng = nc.gpsimd if nt == 0 else nc.vector
        sq = tmp_pool.tile([128, 48], F32, tag="sq", name="sq")
        t2 = tmp_pool.tile([128, 48], F32, tag="t2", name="t2")
        sumsq(eng, sq, t2, qf_coords)
        psum4(eng, qfv[:, :, 41], sq)

        cr = tmp_pool.tile([128, 48], F32, tag="cr", name="cr")
        cs_ = tmp_pool.tile([128, 48], F32, tag="cs_", name="cs_")
        t3 = tmp_pool.tile([128, 48], F32, tag="t3", name="t3")
        first = True
        for (a, b_) in ((1, 2), (2, 0), (0, 1)):
            eng.tensor_tensor(cr[:], qf_dirs[a], comp_qk(nt, b_), OP.mult)
            eng.tensor_tensor(t3[:], qf_dirs[b_], comp_qk(nt, a), OP.mult)
            eng.tensor_tensor(cr[:], cr[:], t3[:], OP.subtract)
            eng.tensor_tensor(cr[:], cr[:], cr[:], OP.mult)
            if first:
                eng.tensor_copy(cs_[:], cr[:])
                first = False
            else:
                eng.tensor_tensor(cs_[:], cs_[:], cr[:], OP.add)
        nq2 = tmp_pool.tile([128, 48], F32, tag="nq2", name="nq2")
        nk2 = tmp_pool.tile([128, 48], F32, tag="nk2", name="nk2")
        sumsq(eng, nq2, t2, qf_dirs)
        sumsq(eng, nk2, t2, [comp_qk(nt, i) for i in range(3)])
        eng.tensor_tensor(nq2[:], nq2[:], nk2[:], OP.mult)
        nc.scalar.activation(nq2[:], nq2[:], AF.Sqrt)
        nc.vector.tensor_scalar_add(nq2[:], nq2[:], EPS)
        nc.vector.reciprocal(nq2[:], nq2[:])
        nc.scalar.activation(cs_[:], cs_[:], AF.Sqrt,
                             scale=float((gw1 / gw0) ** 2))
        eng.tensor_tensor(cs_[:], cs_[:], nq2[:], OP.mult)
        csv = cs_[:].rearrange("p (h x) -> p h x", x=P)
        for x in range(P):
            eng.tensor_tensor(qfv[:, :, 41], qfv[:, :, 41],
                              csv[:, :, x], OP.add)
        eng.tensor_tensor(qfv[:, :, 41], qfv[:, :, 41],
                          qsv[:, :, 40], OP.mult)
        eng.tensor_copy(qfv[:, :, 40], qsv[:, :, 40])

    # warm the EXP table off the critical path
    warm = tmp_pool.tile([2, 2], F32, tag="warm", name="warm")
    nc.vector.memset(warm[:], 0.0)
    nc.scalar.activation(warm[:], warm[:], AF.Exp)

    for nt in range(NT):
        qfv = qf[nt][:].rearrange("p (h f) -> p h f", f=FEAT)
        nc.vector.tensor_tensor(qfv[:, :, 0:40], qfv[:, :, 0:40],
                                qsv[:, :, 0:40], OP.mult)

    qfT = [work.tile([128, NB], BF16, name=f"qfT{t}") for t in range(6)]
    for t in range(6):
        for nt in range(NT):
            ps = tpsum.tile([128, 128], F32, tag="tps")
            nc.tensor.transpose(ps[:], qf[nt][:, t * 128:(t + 1) * 128], ident[:])
            if t % 2:
                nc.scalar.copy(qfT[t][:, nt * 128:(nt + 1) * 128], ps[:])
            else:
                nc.vector.tensor_copy(qfT[t][:, nt * 128:(nt + 1) * 128], ps[:])

    for kt in range(KT):
        ksl = slice(kt * 128, (kt + 1) * 128)
        kfv = kf[kt][:].rearrange("p (h f) -> p h f", f=FEAT)
        vav = va[kt][:].rearrange("p (h f) -> p h f", f=OCH)
        proj_block(sT, ksl, 0, 512,
                   [(0, 192, kfv[:, :, 0:16], "copy"),
                    (192, 384, vav[:, :, 0:16], "copy"),
                    (384, 512, kvp_sb[kt][:, 0:128], "relu")])
        proj_block(sT, ksl, 512, 1024,
                   [(512, 1024, kvp_sb[kt][:, 128:640], "relu")])
        proj_block(sT, ksl, 1024, 1248,
                   [(1024, 1248, kvp_sb[kt][:, 640:864], "relu")])

    for kt in range(KT):
        kfv = kf[kt][:].rearrange("p (h f) -> p h f", f=FEAT)
        vav = va[kt][:].rearrange("p (h f) -> p h f", f=OCH)
        pv_h = kvp_sb[kt][:].rearrange("p (h x c) -> p h x c", x=12, c=6)
        kf_coords = [kfv[:, :, 16 + 8 * i:20 + 8 * i] for i in range(3)]
        kf_dirs = [kfv[:, :, 20 + 8 * i:24 + 8 * i] for i in range(3)]
        emit_tf(lambda j: pv_h[:, :, 0:4, j], lambda j: pv_h[:, :, 0:4, 3 + j],
                kf_coords, kf_dirs, rot_k[kt],
                trn_all[:, kt * 3:(kt + 1) * 3])
        va_coords = [vav[:, :, 16 + 16 * i:24 + 16 * i] for i in range(3)]
        va_dirs = [vav[:, :, 24 + 16 * i:32 + 16 * i] for i in range(3)]
        emit_tf(lambda j: pv_h[:, :, 4:12, j], lambda j: pv_h[:, :, 4:12, 3 + j],
                va_coords, va_dirs, rot_k[kt],
                trn_all[:, kt * 3:(kt + 1) * 3], s1=True)
        # k2: sum of squared coords over pts and comps
        sq = tmp_pool.tile([128, 48], F32, tag="sq", name="sq")
        t2 = tmp_pool.tile([128, 48], F32, tag="t2", name="t2")
        sumsq(nc.gpsimd, sq, t2, kf_coords)
        psum4(nc.gpsimd, kfv[:, :, 40], sq)
        nc.gpsimd.memset(kfv[:, :, 41].bitcast(F32), 1.0)
        nc.gpsimd.memset(vav[:, :, 64], 1.0)
        nc.gpsimd.memset(vav[:, :, 65], 0.0)
        for t in range(6):
            ps = tpsum.tile([128, 128], F32R, tag="tpsr")
            nc.tensor.transpose(ps[:], kf[kt][:, t * 128:(t + 1) * 128],
                                ident_r[:])
            if (kt + t) % 2:
                nc.scalar.copy(kfG[t][:, kt * 128:(kt + 1) * 128],
                               ps[:].bitcast(F32))
            else:
                nc.vector.tensor_copy(kfG[t][:, kt * 128:(kt + 1) * 128],
                                      ps[:].bitcast(F32))
        # fused QK + EXP for this key tile, all heads
        for h in range(H):
            t, base = h // 2, (h % 2) * 64
            aps = qpsum.tile([128, NB], F32, tag="qk", name="aps")
            nc.tensor.matmul(aps[:],
                             kfG[t][base:base + FS, kt * 128:(kt + 1) * 128],
                             qfT[t][base:base + FS, :],
                             start=True, stop=True)
            nc.scalar.activation(
                expT_tiles[h][:, kt * NB:(kt + 1) * NB], aps[:], AF.Exp)

    # ---- inverse transform helper ----------------------------------------
    feats = [work.tile([128, FOUT], F32, name=f"feats{qt}") for qt in range(NT)]
    tinv = [work.tile([128, 3], F32, name=f"tinv{qt}") for qt in range(NT)]
    for qt in range(NT):
        rt, tr = rot_sb[qt], trans_sb[qt]
        for i in range(3):
            nc.vector.tensor_scalar_mul(tinv[qt][:, i:i + 1], tr[:, 0:1],
                                        rt[:, i:i + 1])
            nc.vector.scalar_tensor_tensor(tinv[qt][:, i:i + 1], tr[:, 1:2],
                                           rt[:, 3 + i:4 + i], tinv[qt][:, i:i + 1],
                                           OP.mult, OP.add)
            nc.vector.scalar_tensor_tensor(tinv[qt][:, i:i + 1], tr[:, 2:3],
                                           rt[:, 6 + i:7 + i], tinv[qt][:, i:i + 1],
                                           OP.mult, OP.add)

    def emit_inverse(qt, hh):
        hs = slice(hh * 6, hh * 6 + 6)
        ovv = o_all[qt][:].rearrange("p (h f) -> p h f", f=FEAT)[:, hs]

        def ogp(j):
            return ovv[:, :, 16 + 16 * j:32 + 16 * j]

        gview = feats[qt][:, 192:FOUT].rearrange(
            "p (h x c) -> p h x c", h=H, c=7)[:, hs]
        rt = rot_sb[qt]

        lcld = [tmp_pool.tile([128, 96], F32, tag=f"lcld{i}", name=f"lcld{i}")
                for i in range(3)]
        for i in range(3):
            lv = lcld[i][:].rearrange("p (h x) -> p h x", x=16)
            nc.vector.tensor_scalar_mul(lv, ogp(0), rt[:, i:i + 1])
            nc.vector.scalar_tensor_tensor(lv, ogp(1), rt[:, 3 + i:4 + i],
                                           lv, OP.mult, OP.add)
            nc.vector.scalar_tensor_tensor(lv, ogp(2), rt[:, 6 + i:7 + i],
                                           lv, OP.mult, OP.add)
            nc.vector.tensor_scalar(lv[:, :, 0:8], lv[:, :, 0:8],
                                    tinv[qt][:, i:i + 1], None, OP.subtract)
        n2 = tmp_pool.tile([128, 96], F32, tag="n2", name="n2")
        t2b = tmp_pool.tile([128, 96], F32, tag="t2b", name="t2b")
        nc.gpsimd.tensor_tensor(n2[:], lcld[0][:], lcld[0][:], OP.mult)
        for i in (1, 2):
            nc.gpsimd.tensor_tensor(t2b[:], lcld[i][:], lcld[i][:], OP.mult)
            nc.gpsimd.tensor_tensor(n2[:], n2[:], t2b[:], OP.add)
        n2v = n2[:].rearrange("p (h x) -> p h x", x=16)
        nc.scalar.activation(gview[:, :, :, 6], n2v[:, :, 0:8], AF.Sqrt)
        nc.scalar.activation(n2v[:, :, 8:16], n2v[:, :, 8:16], AF.Sqrt)
        ndv = n2v[:, :, 8:16]
        nc.vector.tensor_scalar_max(ndv, ndv, EPS)
        nc.vector.reciprocal(ndv, ndv)
        for i in range(3):
            lv = lcld[i][:].rearrange("p (h x) -> p h x", x=16)
            nc.gpsimd.tensor_copy(gview[:, :, :, i], lv[:, :, 0:8])
            nc.gpsimd.tensor_tensor(gview[:, :, :, 3 + i],
                                    lv[:, :, 8:16], ndv, OP.mult)

    # ---- attention AV (QK/EXP ran fused in the K-side loop) --------------
    pre_ctx.close()
    pA_ctx.close()
    att_ctx = ExitStack()
    opsum = att_ctx.enter_context(tc.tile_pool(name="opsum", bufs=2, space=PS))
    o_all = [work.tile([128, FEAT * H], F32, name=f"oall{qt}") for qt in range(NT)]

    def emit_av(h):
        expT = expT_tiles[h]
        ot_ps = opsum.tile([OCH, NB], F32, tag="otacc", name="ot_ps")
        for kb in range(NKB):
            nc.tensor.matmul(
                ot_ps[:],
                va[kb][:, h * OCH:(h + 1) * OCH],
                expT[:, kb * NB:(kb + 1) * NB],
                start=(kb == 0), stop=(kb == NKB - 1))
        ot_sb = tmp_pool.tile([OCH, NB], F32R, tag="otsb", name="otsb", bufs=2)
        nc.vector.tensor_copy(ot_sb[:], ot_ps[:])
        for qt in range(NT):
            tp = opsum.tile([128, OCH], F32R, tag="otp", name="tp")
            nc.tensor.transpose(tp[:], ot_sb[:, qt * 128:(qt + 1) * 128],
                                ident_r[0:OCH, 0:OCH])
            rec = tmp_pool.tile([128, 1], F32, tag="rec", name="rec")
            nc.vector.reciprocal(rec[:], tp[:, 64:65].bitcast(F32))
            nc.vector.tensor_scalar_mul(
                o_all[qt][:, h * FEAT + 16:h * FEAT + 64],
                tp[:, 16:64].bitcast(F32), rec[:])
            nc.vector.tensor_scalar_mul(
                feats[qt][:, h * 16:h * 16 + 16],
                tp[:, 0:16].bitcast(F32), rec[:])

    for h in range(H):
        emit_av(h)
        if h == 6:
            for qt in range(NT):
                emit_inverse(qt, 0)
    for qt in range(NT):
        emit_inverse(qt, 1)

    # ---- output projection -----------------------------------------------
    att_ctx.close()
    wout_sb = []
    for kc in range(KCH):
        r0 = kc * 128
        r1 = min(FOUT + 2, r0 + 128)
        t = const.tile([r1 - r0, CS], BF16, name=f"wout{kc}")
        nc.sync.dma_start(t[:], wout_d[r0:r1, :])
        wout_sb.append(t)
    tpsum2 = ctx.enter_context(tc.tile_pool(name="tpsum2", bufs=2, space=PS))
    opsum2 = ctx.enter_context(tc.tile_pool(name="opsum2", bufs=2, space=PS))
    fT = []
    for kc in range(KCH):
        r0 = kc * 128
        rw = min(FOUT, r0 + 128) - r0
        pw = rw + 2 if kc == KCH - 1 else rw
        t = work.tile([pw, NB], BF16, name=f"fT{kc}")
        fT.append(t)
    lastr = FOUT - (KCH - 1) * 128
    nc.vector.tensor_copy(fT[KCH - 1][lastr:lastr + 2, :], ones2_f32[:])
    for kc in range(KCH):
        r0 = kc * 128
        rw = min(FOUT, r0 + 128) - r0
        for qt in range(NT):
            ps = tpsum2.tile([128, 128], F32, tag="tps2")
            nc.tensor.transpose(ps[:rw, :], feats[qt][:, r0:r0 + rw], ident[:])
            if kc % 2:
                nc.scalar.copy(fT[kc][:rw, qt * 128:(qt + 1) * 128], ps[:rw, :])
            else:
                nc.vector.tensor_copy(fT[kc][:rw, qt * 128:(qt + 1) * 128],
                                      ps[:rw, :])

    for qt in range(NT):
        ps = opsum2.tile([128, CS], F32, tag="oproj")
        for kc in range(KCH):
            nc.tensor.matmul(ps[:], fT[kc][:, qt * 128:(qt + 1) * 128],
                             wout_sb[kc][:], start=(kc == 0), stop=(kc == KCH - 1))
        osb = tmp_pool.tile([128, CS], F32, tag="osb", name="osb")
        nc.scalar.copy(osb[:], ps[:])
        nc.sync.dma_start(out_loc[qt * 128:(qt + 1) * 128, :], osb[:])


def _run(inputs, trace=False):
    (s, sT, rot9, trans, wall, wout_b, qscale, bfbits, gw) = _host_prep(inputs)
    nc = _build_program(float(gw[0]), float(gw[1]))
    in_maps = []
    for c in range(8):
        b, qb = c // 4, c % 4
        r = slice(qb * NB, (qb + 1) * NB)
        in_maps.append({
            "sT_d": bfbits(sT[b]),
            "qsT_d": bfbits(sT[b][:, r]),
            "rot_full": np.ascontiguousarray(rot9[b]),
            "trans_full": np.ascontiguousarray(trans[b]),
            "q_rot": np.ascontiguousarray(rot9[b, r]),
            "q_trans": np.ascontiguousarray(trans[b, r]),
            "wall": wall, "wout_b": wout_b, "qscale": qscale,
        })
    res = run_bass_kernel_spmd(nc, in_maps, list(range(8)), trace=trace)
    out = np.empty((B, N, CS), np.float32)
    for c in range(8):
        b, qb = c // 4, c % 4
        out[b, qb * NB:(qb + 1) * NB] = res.results[c]["out_loc"]
    return out, res


def kernel(**inputs):
    out, _ = _run(inputs, trace=False)
    return out


def kernel_traced(**inputs):
    return _run(inputs, trace=True)
